# revision 19
# baseline (speedup 1.0000x reference)
"""Multi-head attention (B=1, L=4096, D=512, H=8, DH=64) on 8 TRN2 NeuronCores.

Sharding: head-parallel - core h computes head h end-to-end; host reduces
partial y over heads.

Key device-side structure (v2):
  - Scores are computed transposed (ST[j, i] = k_j . q_i) with k pre-scaled
    by log2e/256 so the stored score s equals w with exp(qk/8) = (2^w)^32.
  - exp runs on BOTH ScalarE (ACT Exp, scale=32*ln2) and the DVE via two
    custom DVE ops (deg-3 poly of 2^w with c0=1, then 5 squarings); a
    fraction of groups is offloaded to DVE to relieve the ScalarE
    bottleneck (~151us of ACT in the previous version).
  - PV is "swapped": pt (the exp'd scores, [128 keys, 512 q]) is the
    stationary operand in 128-col q-chunks and vext [128 keys, 65] moves,
    so each j-tile streams 4x65 columns instead of 512 - about half the
    PE stream time of the classic orientation. Output lands [q, 4, 65]
    per-partition-query, so softmax normalization is a per-partition
    reciprocal+scale (no cross-partition broadcast at all).
  - The normalized [q, dh] tile is DMA-transposed (xbar engine, otherwise
    idle) into [dh, q] for the output projection.
  - k is projected straight into parity placement (even j-tiles in
    partitions 0:64, odd in 64:128 - the other half was never read in the
    dup scheme), q is projected once and row-duplicated with an SBUF DMA;
    both halve their projection matmul cost.
  - x is DMA'd in 8 per-i-tile chunks (separate tiles) so the first
    projection only waits on chunk 0, not the full 4MB load.
"""

import os

import numpy as np

import concourse.bass as bass
import concourse.mybir as mybir
import concourse.tile as tile
from concourse import bacc, dve_ops
from concourse.bass import ts
from concourse.dve_spec import Spec, Src0, C0, C1, C2, One, sq, lower as dve_lower
from concourse.dve_uop import DveOpSpec

F32 = mybir.dt.float32
F16 = mybir.dt.float16

L = 4096  # sequence length
D = 512  # model dim
H = 8  # heads
DH = 64  # head dim
P = 128  # partitions
DC = D // P  # d-chunks for the projection contraction (4)
IW = 512  # i-tile (query) width
NI = L // IW  # 8
NJ = L // P  # 32 j-tiles (key blocks)
GJ = 2  # j-tiles per exp group (2 PSUM banks per exp instruction)
NG = NJ // GJ  # groups per i-tile (16)
WCOLS = 320  # q-dup(128) + k-dup(128) + v(64)
N_CORES = 8

K_PRESCALE = float(np.log2(np.e) / 256.0)  # folded into Wk on the host
ACT_SCALE = float(32.0 * np.log(2.0))  # exp(score/8) = exp(w * 32 ln2)
# p(w) ~ 2^w on [-0.35, 0.35] with c0 = 1; exp(score/8) = p^32
C_P1 = 0.69316309
C_P2 = 0.24113793
C_P3 = 0.05545293

SWAPPED_PV = False  # pt stationary / v moving (else classic v^T @ pt)
# NOTE: swapped orientation is broken on HW - interleaved accumulation
# chains within one PSUM bank corrupt results (verified by micro-test);
# 4 separate banks don't fit the 8-bank budget alongside stp/proj.
DVE_EXP = True  # offload some exp groups to the DVE custom ops
DVE_PHASE = 2  # groups with g % 4 == DVE_PHASE run exp on the DVE
PV_LAG = 2  # groups between exp(g) and the PV matmuls reading pt(g)

_CACHE = {}
LAST = {}


def _register_dve(name, spec_body, reference):
    spec = Spec(body=spec_body, reference=reference)
    row = dve_ops._SUB_OPCODE_FOR_NAME.get(name)
    if row is None:
        row = max(dve_ops._SUB_OPCODE_FOR_NAME.values()) + 1
        assert row < 0x20
        dve_ops._SUB_OPCODE_FOR_NAME[name] = row
    for prev in dve_ops.OPS:
        if prev.name == name:
            return prev
    shas = {}
    for ver in ("v3", "v4"):
        s = DveOpSpec(
            name=name, opcode=row, uops=dve_lower(spec, ver=ver), rd1_en=False
        )
        shas[ver] = s.sha(ver)
    op = dve_ops.DveOp(name, spec, subdim=False, uops_sha=shas)
    dve_ops.OPS.append(op)
    dve_ops.CUSTOM_DVE_SPECS[name] = spec
    return op


def _ref_poly(in0, in1, s0, s1, imm2):
    w = in0.astype(np.float32)
    return (1.0 + w * (s0 + w * (s1 + w * imm2))).astype(np.float32)


def _ref_sq32(in0, in1, s0, s1, imm2):
    p = in0.astype(np.float32)
    for _ in range(5):
        p = p * p
    return p


EXP2_POLY = _register_dve(
    "EXP2_POLY_ANT1", One + Src0 * (C0 + Src0 * (C1 + Src0 * C2)), _ref_poly
)
EXP2_SQ32 = _register_dve(
    "EXP2_SQ32_ANT1", sq(sq(sq(sq(sq(Src0))))), _ref_sq32
)


def build_bass():
    nc = bacc.Bacc(
        "TRN2", target_bir_lowering=False, debug=False, num_devices=N_CORES
    )
    xt = nc.dram_tensor("xt", [NI, DC, P, IW], F16, kind="ExternalInput")
    w = nc.dram_tensor("w", [DC, P, WCOLS], F16, kind="ExternalInput")
    wo = nc.dram_tensor("wo", [DH, D], F16, kind="ExternalInput")
    y = nc.dram_tensor("y", [L // P, P, D], F32, kind="ExternalOutput")

    with (
        tile.TileContext(nc) as tc,
        tc.tile_pool(name="const", bufs=1) as cpool,
        tc.tile_pool(name="ps", bufs=1, space="PSUM") as ppool,
        tc.tile_pool(name="pt", bufs=1) as pt_pool,
        tc.tile_pool(name="post", bufs=1) as post_pool,
    ):
        x_sbs = [cpool.tile([P, DC, IW], F16, name=f"x{i}") for i in range(NI)]
        w_sb = cpool.tile([P, DC, WCOLS], F16)
        wo_sb = cpool.tile([DH, D], F16)
        nc.sync.dma_start(x_sbs[0][:], xt[0].rearrange("c p l -> p c l"))
        nc.sync.dma_start(w_sb[:], w.rearrange("c p m -> p c m"))
        nc.sync.dma_start(wo_sb[:], wo[:])
        for i in range(1, NI):
            nc.sync.dma_start(x_sbs[i][:], xt[i].rearrange("c p l -> p c l"))

        qdup = cpool.tile([P, L], F16)  # qT in rows 0:64 AND 64:128
        kdup = cpool.tile([P, L], F16)
        vext = cpool.tile([P, NJ, DH + 2], F16)
        nc.vector.memset(vext[:, :, DH], 1.0)
        # warm the ACT exp table while DMAs run
        warm = cpool.tile([1, 8], F32)
        nc.vector.memset(warm[:], 0.0)
        nc.scalar.activation(warm[:], warm[:], mybir.ActivationFunctionType.Exp)

        def emit_proj_kq(i2):
            # k first (gates the score j-tiles), then q
            for off, dst in ((P, kdup), (0, qdup)):
                ps = ppool.tile([P, IW], F32, tag="proj", bufs=2, name="ps")
                for c in range(DC):
                    nc.tensor.matmul(
                        ps[:],
                        lhsT=w_sb[:, c, off : off + P],
                        rhs=x_sbs[i2][:, c, :],
                        start=(c == 0),
                        stop=(c == DC - 1),
                    )
                nc.vector.tensor_copy(dst[:, ts(i2, IW)], ps[:])

        def emit_proj_v(i2):
            for t in range(4 * i2, 4 * i2 + 4):
                psv = ppool.tile([P, DH], F32, tag="proj", bufs=2, name="psv")
                for c in range(DC):
                    nc.tensor.matmul(
                        psv[:],
                        lhsT=x_sbs[i2][:, c, ts(t % 4, P)],
                        rhs=w_sb[:, c, 2 * P : 2 * P + DH],
                        start=(c == 0),
                        stop=(c == DC - 1),
                    )
                nc.vector.tensor_copy(vext[:, t, 0:DH], psv[:])

        pvs = {}
        pts = {}
        o_norms = {}
        outTs = {}

        def emit_scores_exp(i, g, dve_ok):
            stp = ppool.tile([P, GJ, IW], F32, tag="st", bufs=2, name="stp")
            for u in range(GJ):
                jt = g * GJ + u
                half = DH * (jt % 2)
                nc.tensor.matmul(
                    stp[:, u, :],
                    lhsT=kdup[half : half + DH, ts(jt, P)],
                    rhs=qdup[half : half + DH, ts(i, IW)],
                    start=True,
                    stop=True,
                )
            pt = pt_pool.tile([P, GJ, IW], F16, tag="pt", bufs=24, name="pt")
            if DVE_EXP and dve_ok and g % 4 == DVE_PHASE:
                scr = pt_pool.tile(
                    [P, GJ * IW], F32, tag="scr", bufs=2, name="scr"
                )
                nc.vector._custom_dve(
                    EXP2_POLY,
                    out=scr[:],
                    in0=stp[:, :, :],
                    s0=C_P1,
                    s1=C_P2,
                    imm2=C_P3,
                )
                nc.vector._custom_dve(
                    EXP2_SQ32,
                    out=pt[:, :, :],
                    in0=scr[:],
                )
            else:
                nc.scalar.activation(
                    pt[:, :, :],
                    stp[:, :, :],
                    mybir.ActivationFunctionType.Exp,
                    scale=ACT_SCALE,
                )
            pts[(i, g)] = pt

        def emit_pv(i, g):
            if g == 0:
                if SWAPPED_PV:
                    pvs[i] = ppool.tile(
                        [P, 4, DH + 1], F32, tag="acc", bufs=2, name=f"pv{i}"
                    )
                else:
                    pvs[i] = ppool.tile(
                        [DH + 1, IW], F32, tag="acc", bufs=2, name=f"pv{i}"
                    )
            pt = pts.pop((i, g))
            for u in range(GJ):
                jt = g * GJ + u
                if SWAPPED_PV:
                    for c in range(4):
                        nc.tensor.matmul(
                            pvs[i][:, c, :],
                            lhsT=pt[:, u, ts(c, P)],
                            rhs=vext[:, jt, 0 : DH + 1],
                            start=(jt == 0),
                            stop=(jt == NJ - 1),
                            skip_group_check=True,
                        )
                else:
                    nc.tensor.matmul(
                        pvs[i][:],
                        lhsT=vext[:, jt, 0 : DH + 1],
                        rhs=pt[:, u, :],
                        start=(jt == 0),
                        stop=(jt == NJ - 1),
                        skip_group_check=True,
                    )

        def emit_post_head(i):
            pv = pvs[i]
            # stage pv to SBUF once (frees PSUM + lets GpSimd work on it)
            pv_sb = post_pool.tile(
                [DH + 1, IW], F32, tag="pvsb", bufs=2, name="pvsb"
            )
            nc.vector.tensor_copy(pv_sb[:], pv[:])
            srow = post_pool.tile([1, IW], F32, tag="srow", bufs=2, name="srow")
            nc.vector.tensor_copy(srow[:], pv_sb[DH : DH + 1, :])
            rcp = post_pool.tile([1, IW], F32, tag="rcp", bufs=2, name="rcp")
            nc.vector.reciprocal_approx_fast(rcp[:], srow[:])
            rb = post_pool.tile([DH, IW], F32, tag="rb", bufs=2, name="rb")
            nc.gpsimd.partition_broadcast(rb[:], rcp[:])
            outT = post_pool.tile([DH, IW], F16, tag="outT", bufs=2, name="outT")
            nc.gpsimd.tensor_mul(outT[:], pv_sb[0:DH, :], rb[:])
            outTs[i] = outT

        def emit_post_y(i, t):
            yps = ppool.tile([P, D], F32, tag="proj", bufs=2, name="yps")
            nc.tensor.matmul(
                yps[:],
                lhsT=outTs[i][:, ts(t, P)],
                rhs=wo_sb[:],
                start=True,
                stop=True,
            )
            ysb = post_pool.tile([P, D], F32, tag="ysb", bufs=3, name="ysb")
            nc.vector.tensor_copy(ysb[:], yps[:])
            nc.sync.dma_start(y[i * (IW // P) + t], ysb[:])

        # --- emission with PV lag ---
        from collections import deque

        pending = deque()

        def pump():
            if pending:
                pending.popleft()()

        pv_queue = deque()  # (i, g) whose PV matmuls still need emitting
        pv_defer = []  # i-tile 2's PVs wait for post_head(0) to free a pv slot

        def group(i, g, dve_ok=False, defer_pv=False):
            emit_scores_exp(i, g, dve_ok)
            if defer_pv:
                pv_defer.append((i, g))
                return
            pv_queue.append((i, g))
            while len(pv_queue) > PV_LAG:
                emit_pv(*pv_queue.popleft())

        # --- prologue: projections interleaved with i-tiles 0..2 ---
        for i2 in range(NI):
            emit_proj_kq(i2)
            emit_proj_v(i2)
            group(0, 2 * i2)
            group(0, 2 * i2 + 1)
            if i2 > 0:
                group(1, 2 * (i2 - 1))
                group(1, 2 * (i2 - 1) + 1)
            if i2 > 1:
                group(2, 2 * (i2 - 2), defer_pv=True)
                group(2, 2 * (i2 - 2) + 1, defer_pv=True)
        group(1, NG - 2)
        group(1, NG - 1)
        for g in range(2 * (NI - 2), NG):
            group(2, g, defer_pv=True)
        while pv_queue:
            emit_pv(*pv_queue.popleft())  # finish PV(0,*) and PV(1,*)
        # post i-tile 0 frees a pv slot, then i-tile 2's deferred PVs run
        emit_post_head(0)
        for t in range(IW // P):
            pending.append(lambda t=t: emit_post_y(0, t))
        for ig in pv_defer[:2]:
            emit_pv(*ig)
        pump()
        for ig in pv_defer[2:]:
            emit_pv(*ig)
        pv_defer.clear()
        for i in (1, 2):
            pending.append(lambda i=i: emit_post_head(i))
            for t in range(IW // P):
                pending.append(lambda i=i, t=t: emit_post_y(i, t))
        # --- steady state ---
        for i in range(3, NI):
            for g in range(NG):
                group(i, g, dve_ok=True)
                if g % 3 == 1:
                    pump()
            pending.append(lambda i=i: emit_post_head(i))
            for t in range(IW // P):
                pending.append(lambda i=i, t=t: emit_post_y(i, t))
        while pv_queue:
            emit_pv(*pv_queue.popleft())
        while pending:
            pump()
    nc.compile()
    return nc


def _get_nc():
    if "nc" not in _CACHE:
        _CACHE["nc"] = build_bass()
    return _CACHE["nc"]


def _prep_in_maps(x, Wqkv, Wo):
    x = np.asarray(x, dtype=np.float32).reshape(L, D)
    Wqkv = np.asarray(Wqkv, dtype=np.float32)
    Wo = np.asarray(Wo, dtype=np.float32)
    xt = (
        np.ascontiguousarray(x.T)
        .reshape(DC, P, NI, IW)
        .transpose(2, 0, 1, 3)
        .astype(np.float16)
    )
    xt = np.ascontiguousarray(xt)
    in_maps = []
    for h in range(N_CORES):
        wq = Wqkv[:, 0 * D + h * DH : 0 * D + (h + 1) * DH]
        wk = Wqkv[:, 1 * D + h * DH : 1 * D + (h + 1) * DH] * K_PRESCALE
        wv = Wqkv[:, 2 * D + h * DH : 2 * D + (h + 1) * DH]
        cols = np.concatenate([wq, wq, wk, wk, wv], axis=1)  # [512, 320]
        w_dram = np.ascontiguousarray(cols).reshape(DC, P, WCOLS).astype(np.float16)
        wo_h = np.ascontiguousarray(Wo[h * DH : (h + 1) * DH, :]).astype(np.float16)
        in_maps.append({"xt": xt, "w": w_dram, "wo": wo_h})
    return in_maps


def kernel(x, Wqkv, Wo):
    from concourse import bass_utils

    # zero-egress container: artifact upload is impossible and only feeds
    # trace metadata - replace with a local marker.
    bass_utils.upload_artifacts = lambda tmpdir: f"local://{tmpdir}"

    nc = _get_nc()
    in_maps = _prep_in_maps(x, Wqkv, Wo)
    trace = bool(os.environ.get("KERNEL_TRACE"))
    res = bass_utils.run_bass_kernel_spmd(
        nc, in_maps, core_ids=list(range(N_CORES)), trace=trace
    )
    LAST["exec_time_ns"] = res.exec_time_ns
    LAST["trace"] = res.instructions_and_trace
    acc = np.zeros((L, D), np.float32)
    for r in res.results:
        acc += r["y"].reshape(L, D)
    return acc.reshape(1, L, D).astype(np.float32)


# revision 20
# speedup vs baseline: 1.2838x; 1.2838x over previous
"""Multi-head attention (B=1, L=4096, D=512, H=8, DH=64) on 8 TRN2 NeuronCores.

Sharding: head-parallel - core h computes head h end-to-end; host reduces
partial y over heads.

Key device-side structure (v2):
  - Scores are computed transposed (ST[j, i] = k_j . q_i) with k pre-scaled
    by log2e/256 so the stored score s equals w with exp(qk/8) = (2^w)^32.
  - exp runs on BOTH ScalarE (ACT Exp, scale=32*ln2) and the DVE via two
    custom DVE ops (deg-3 poly of 2^w with c0=1, then 5 squarings); a
    fraction of groups is offloaded to DVE to relieve the ScalarE
    bottleneck (~151us of ACT in the previous version).
  - PV is "swapped": pt (the exp'd scores, [128 keys, 512 q]) is the
    stationary operand in 128-col q-chunks and vext [128 keys, 65] moves,
    so each j-tile streams 4x65 columns instead of 512 - about half the
    PE stream time of the classic orientation. Output lands [q, 4, 65]
    per-partition-query, so softmax normalization is a per-partition
    reciprocal+scale (no cross-partition broadcast at all).
  - The normalized [q, dh] tile is DMA-transposed (xbar engine, otherwise
    idle) into [dh, q] for the output projection.
  - k is projected straight into parity placement (even j-tiles in
    partitions 0:64, odd in 64:128 - the other half was never read in the
    dup scheme), q is projected once and row-duplicated with an SBUF DMA;
    both halve their projection matmul cost.
  - x is DMA'd in 8 per-i-tile chunks (separate tiles) so the first
    projection only waits on chunk 0, not the full 4MB load.
"""

import os

import numpy as np

import concourse.bass as bass
import concourse.mybir as mybir
import concourse.tile as tile
from concourse import bacc, dve_ops
from concourse.bass import ts
from concourse.dve_spec import Spec, Src0, C0, C1, C2, One, sq, lower as dve_lower
from concourse.dve_uop import DveOpSpec

F32 = mybir.dt.float32
F16 = mybir.dt.float16

L = 4096  # sequence length
D = 512  # model dim
H = 8  # heads
DH = 64  # head dim
P = 128  # partitions
DC = D // P  # d-chunks for the projection contraction (4)
IW = 512  # i-tile (query) width
NI = L // IW  # 8
NJ = L // P  # 32 j-tiles (key blocks)
GJ = 2  # j-tiles per exp group (2 PSUM banks per exp instruction)
NG = NJ // GJ  # groups per i-tile (16)
WCOLS = 320  # q-dup(128) + k-dup(128) + v(64)
N_CORES = 8

K_PRESCALE = float(np.log2(np.e) / 256.0)  # folded into Wk on the host
ACT_SCALE = float(32.0 * np.log(2.0))  # exp(score/8) = exp(w * 32 ln2)
# p(w) ~ 2^w on [-0.35, 0.35] with c0 = 1; exp(score/8) = p^32
C_P1 = 0.69316309
C_P2 = 0.24113793
C_P3 = 0.05545293

SWAPPED_PV = False  # pt stationary / v moving (else classic v^T @ pt)
# NOTE: swapped orientation is broken on HW - interleaved accumulation
# chains within one PSUM bank corrupt results (verified by micro-test);
# 4 separate banks don't fit the 8-bank budget alongside stp/proj.
DVE_EXP = False  # offload some exp groups to the DVE custom ops
DVE_PHASE = 2  # groups with g % 4 == DVE_PHASE run exp on the DVE
PV_LAG = 2  # groups between exp(g) and the PV matmuls reading pt(g)

_CACHE = {}
LAST = {}


def _register_dve(name, spec_body, reference):
    spec = Spec(body=spec_body, reference=reference)
    row = dve_ops._SUB_OPCODE_FOR_NAME.get(name)
    if row is None:
        row = max(dve_ops._SUB_OPCODE_FOR_NAME.values()) + 1
        assert row < 0x20
        dve_ops._SUB_OPCODE_FOR_NAME[name] = row
    for prev in dve_ops.OPS:
        if prev.name == name:
            return prev
    shas = {}
    for ver in ("v3", "v4"):
        s = DveOpSpec(
            name=name, opcode=row, uops=dve_lower(spec, ver=ver), rd1_en=False
        )
        shas[ver] = s.sha(ver)
    op = dve_ops.DveOp(name, spec, subdim=False, uops_sha=shas)
    dve_ops.OPS.append(op)
    dve_ops.CUSTOM_DVE_SPECS[name] = spec
    return op


def _ref_poly(in0, in1, s0, s1, imm2):
    w = in0.astype(np.float32)
    return (1.0 + w * (s0 + w * (s1 + w * imm2))).astype(np.float32)


def _ref_sq32(in0, in1, s0, s1, imm2):
    p = in0.astype(np.float32)
    for _ in range(5):
        p = p * p
    return p


EXP2_POLY = _register_dve(
    "EXP2_POLY_ANT1", One + Src0 * (C0 + Src0 * (C1 + Src0 * C2)), _ref_poly
)
EXP2_SQ32 = _register_dve(
    "EXP2_SQ32_ANT1", sq(sq(sq(sq(sq(Src0))))), _ref_sq32
)


def build_bass():
    nc = bacc.Bacc(
        "TRN2", target_bir_lowering=False, debug=False, num_devices=N_CORES
    )
    xt = nc.dram_tensor("xt", [NI, DC, P, IW], F16, kind="ExternalInput")
    w = nc.dram_tensor("w", [DC, P, WCOLS], F16, kind="ExternalInput")
    wo = nc.dram_tensor("wo", [DH, D], F16, kind="ExternalInput")
    y = nc.dram_tensor("y", [L // P, P, D], F32, kind="ExternalOutput")

    with (
        tile.TileContext(nc) as tc,
        tc.tile_pool(name="const", bufs=1) as cpool,
        tc.tile_pool(name="ps", bufs=1, space="PSUM") as ppool,
        tc.tile_pool(name="pt", bufs=1) as pt_pool,
        tc.tile_pool(name="post", bufs=1) as post_pool,
    ):
        x_sbs = [cpool.tile([P, DC, IW], F16, name=f"x{i}") for i in range(NI)]
        w_sb = cpool.tile([P, DC, WCOLS], F16)
        wo_sb = cpool.tile([DH, D], F16)
        nc.sync.dma_start(x_sbs[0][:], xt[0].rearrange("c p l -> p c l"))
        nc.sync.dma_start(w_sb[:], w.rearrange("c p m -> p c m"))
        nc.sync.dma_start(wo_sb[:], wo[:])
        for i in range(1, NI):
            nc.sync.dma_start(x_sbs[i][:], xt[i].rearrange("c p l -> p c l"))

        qdup = cpool.tile([P, L], F16)  # qT in rows 0:64 AND 64:128
        kdup = cpool.tile([P, L], F16)
        vext = cpool.tile([P, NJ, DH + 2], F16)
        nc.vector.memset(vext[:, :, DH], 1.0)
        # warm the ACT exp table while DMAs run
        warm = cpool.tile([1, 8], F32)
        nc.vector.memset(warm[:], 0.0)
        nc.scalar.activation(warm[:], warm[:], mybir.ActivationFunctionType.Exp)

        def emit_proj_kq(i2):
            # k first (gates the score j-tiles), then q
            for off, dst in ((P, kdup), (0, qdup)):
                ps = ppool.tile([P, IW], F32, tag="proj", bufs=2, name="ps")
                for c in range(DC):
                    nc.tensor.matmul(
                        ps[:],
                        lhsT=w_sb[:, c, off : off + P],
                        rhs=x_sbs[i2][:, c, :],
                        start=(c == 0),
                        stop=(c == DC - 1),
                    )
                nc.vector.tensor_copy(dst[:, ts(i2, IW)], ps[:])

        def emit_proj_v(i2):
            for t in range(4 * i2, 4 * i2 + 4):
                psv = ppool.tile([P, DH], F32, tag="proj", bufs=2, name="psv")
                for c in range(DC):
                    nc.tensor.matmul(
                        psv[:],
                        lhsT=x_sbs[i2][:, c, ts(t % 4, P)],
                        rhs=w_sb[:, c, 2 * P : 2 * P + DH],
                        start=(c == 0),
                        stop=(c == DC - 1),
                    )
                nc.vector.tensor_copy(vext[:, t, 0:DH], psv[:])

        pvs = {}
        pts = {}
        o_norms = {}
        outTs = {}

        def emit_scores_exp(i, g, dve_ok):
            stp = ppool.tile([P, GJ, IW], F32, tag="st", bufs=2, name="stp")
            for u in range(GJ):
                jt = g * GJ + u
                half = DH * (jt % 2)
                nc.tensor.matmul(
                    stp[:, u, :],
                    lhsT=kdup[half : half + DH, ts(jt, P)],
                    rhs=qdup[half : half + DH, ts(i, IW)],
                    start=True,
                    stop=True,
                )
            pt = pt_pool.tile([P, GJ, IW], F16, tag="pt", bufs=24, name="pt")
            if DVE_EXP and dve_ok and g % 4 == DVE_PHASE:
                scr = pt_pool.tile(
                    [P, GJ * IW], F32, tag="scr", bufs=2, name="scr"
                )
                nc.vector._custom_dve(
                    EXP2_POLY,
                    out=scr[:],
                    in0=stp[:, :, :],
                    s0=C_P1,
                    s1=C_P2,
                    imm2=C_P3,
                )
                nc.vector._custom_dve(
                    EXP2_SQ32,
                    out=pt[:, :, :],
                    in0=scr[:],
                )
            else:
                nc.scalar.activation(
                    pt[:, :, :],
                    stp[:, :, :],
                    mybir.ActivationFunctionType.Exp,
                    scale=ACT_SCALE,
                )
            pts[(i, g)] = pt

        def emit_pv(i, g):
            if g == 0:
                if SWAPPED_PV:
                    pvs[i] = ppool.tile(
                        [P, 4, DH + 1], F32, tag="acc", bufs=2, name=f"pv{i}"
                    )
                else:
                    pvs[i] = ppool.tile(
                        [DH + 1, IW], F32, tag="acc", bufs=2, name=f"pv{i}"
                    )
            pt = pts.pop((i, g))
            for u in range(GJ):
                jt = g * GJ + u
                if SWAPPED_PV:
                    for c in range(4):
                        nc.tensor.matmul(
                            pvs[i][:, c, :],
                            lhsT=pt[:, u, ts(c, P)],
                            rhs=vext[:, jt, 0 : DH + 1],
                            start=(jt == 0),
                            stop=(jt == NJ - 1),
                            skip_group_check=True,
                        )
                else:
                    nc.tensor.matmul(
                        pvs[i][:],
                        lhsT=vext[:, jt, 0 : DH + 1],
                        rhs=pt[:, u, :],
                        start=(jt == 0),
                        stop=(jt == NJ - 1),
                        skip_group_check=True,
                    )

        def emit_post_head(i):
            pv = pvs[i]
            # stage pv to SBUF once (frees PSUM + lets GpSimd work on it)
            pv_sb = post_pool.tile(
                [DH + 1, IW], F32, tag="pvsb", bufs=2, name="pvsb"
            )
            nc.vector.tensor_copy(pv_sb[:], pv[:])
            srow = post_pool.tile([1, IW], F32, tag="srow", bufs=2, name="srow")
            nc.vector.tensor_copy(srow[:], pv_sb[DH : DH + 1, :])
            rcp = post_pool.tile([1, IW], F32, tag="rcp", bufs=2, name="rcp")
            nc.vector.reciprocal_approx_fast(rcp[:], srow[:])
            rb = post_pool.tile([DH, IW], F32, tag="rb", bufs=2, name="rb")
            nc.gpsimd.partition_broadcast(rb[:], rcp[:])
            outT = post_pool.tile([DH, IW], F16, tag="outT", bufs=2, name="outT")
            nc.gpsimd.tensor_mul(outT[:], pv_sb[0:DH, :], rb[:])
            outTs[i] = outT

        def emit_post_y(i, t):
            yps = ppool.tile([P, D], F32, tag="proj", bufs=2, name="yps")
            nc.tensor.matmul(
                yps[:],
                lhsT=outTs[i][:, ts(t, P)],
                rhs=wo_sb[:],
                start=True,
                stop=True,
            )
            ysb = post_pool.tile([P, D], F32, tag="ysb", bufs=3, name="ysb")
            nc.vector.tensor_copy(ysb[:], yps[:])
            nc.sync.dma_start(y[i * (IW // P) + t], ysb[:])

        # --- emission with PV lag ---
        from collections import deque

        pending = deque()

        def pump():
            if pending:
                pending.popleft()()

        pv_queue = deque()  # (i, g) whose PV matmuls still need emitting
        pv_defer = []  # i-tile 2's PVs wait for post_head(0) to free a pv slot

        def group(i, g, dve_ok=False, defer_pv=False):
            emit_scores_exp(i, g, dve_ok)
            if defer_pv:
                pv_defer.append((i, g))
                return
            pv_queue.append((i, g))
            while len(pv_queue) > PV_LAG:
                emit_pv(*pv_queue.popleft())

        # --- prologue: projections interleaved with i-tiles 0..2 ---
        for i2 in range(NI):
            emit_proj_kq(i2)
            emit_proj_v(i2)
            group(0, 2 * i2)
            group(0, 2 * i2 + 1)
            if i2 > 0:
                group(1, 2 * (i2 - 1))
                group(1, 2 * (i2 - 1) + 1)
            if i2 > 1:
                group(2, 2 * (i2 - 2), defer_pv=True)
                group(2, 2 * (i2 - 2) + 1, defer_pv=True)
        group(1, NG - 2)
        group(1, NG - 1)
        for g in range(2 * (NI - 2), NG):
            group(2, g, defer_pv=True)
        while pv_queue:
            emit_pv(*pv_queue.popleft())  # finish PV(0,*) and PV(1,*)
        # post i-tile 0 frees a pv slot, then i-tile 2's deferred PVs run
        emit_post_head(0)
        for t in range(IW // P):
            pending.append(lambda t=t: emit_post_y(0, t))
        for ig in pv_defer[:2]:
            emit_pv(*ig)
        pump()
        for ig in pv_defer[2:]:
            emit_pv(*ig)
        pv_defer.clear()
        for i in (1, 2):
            pending.append(lambda i=i: emit_post_head(i))
            for t in range(IW // P):
                pending.append(lambda i=i, t=t: emit_post_y(i, t))
        # --- steady state ---
        for i in range(3, NI):
            for g in range(NG):
                group(i, g, dve_ok=True)
                if g % 3 == 1:
                    pump()
            pending.append(lambda i=i: emit_post_head(i))
            for t in range(IW // P):
                pending.append(lambda i=i, t=t: emit_post_y(i, t))
        while pv_queue:
            emit_pv(*pv_queue.popleft())
        while pending:
            pump()
    nc.compile()
    return nc


def _get_nc():
    if "nc" not in _CACHE:
        _CACHE["nc"] = build_bass()
    return _CACHE["nc"]


def _prep_in_maps(x, Wqkv, Wo):
    x = np.asarray(x, dtype=np.float32).reshape(L, D)
    Wqkv = np.asarray(Wqkv, dtype=np.float32)
    Wo = np.asarray(Wo, dtype=np.float32)
    xt = (
        np.ascontiguousarray(x.T)
        .reshape(DC, P, NI, IW)
        .transpose(2, 0, 1, 3)
        .astype(np.float16)
    )
    xt = np.ascontiguousarray(xt)
    in_maps = []
    for h in range(N_CORES):
        wq = Wqkv[:, 0 * D + h * DH : 0 * D + (h + 1) * DH]
        wk = Wqkv[:, 1 * D + h * DH : 1 * D + (h + 1) * DH] * K_PRESCALE
        wv = Wqkv[:, 2 * D + h * DH : 2 * D + (h + 1) * DH]
        cols = np.concatenate([wq, wq, wk, wk, wv], axis=1)  # [512, 320]
        w_dram = np.ascontiguousarray(cols).reshape(DC, P, WCOLS).astype(np.float16)
        wo_h = np.ascontiguousarray(Wo[h * DH : (h + 1) * DH, :]).astype(np.float16)
        in_maps.append({"xt": xt, "w": w_dram, "wo": wo_h})
    return in_maps


def kernel(x, Wqkv, Wo):
    from concourse import bass_utils

    # zero-egress container: artifact upload is impossible and only feeds
    # trace metadata - replace with a local marker.
    bass_utils.upload_artifacts = lambda tmpdir: f"local://{tmpdir}"

    nc = _get_nc()
    in_maps = _prep_in_maps(x, Wqkv, Wo)
    trace = bool(os.environ.get("KERNEL_TRACE"))
    res = bass_utils.run_bass_kernel_spmd(
        nc, in_maps, core_ids=list(range(N_CORES)), trace=trace
    )
    LAST["exec_time_ns"] = res.exec_time_ns
    LAST["trace"] = res.instructions_and_trace
    acc = np.zeros((L, D), np.float32)
    for r in res.results:
        acc += r["y"].reshape(L, D)
    return acc.reshape(1, L, D).astype(np.float32)


# revision 26
# speedup vs baseline: 1.5170x; 1.1817x over previous
"""Multi-head attention (B=1, L=4096, D=512, H=8, DH=64) on 8 TRN2 NeuronCores.

Sharding: head-parallel — core h computes head h end-to-end:
    qkv_h = x @ Wqkv[:, head-slices]      (on device, from host-transposed x)
    attn_h = softmax(q k^T / 8) v          (transposed-score layout)
    y_h = attn_h @ Wo[h*64:(h+1)*64, :]    (partial over heads)
Host reduces: y = sum_h y_h.

Device layout notes:
  - All score tiles are computed transposed: ST[j, i] = k_j . q_i, so the
    P@V contraction (over j) can use PT directly as the matmul moving
    operand. Softmax denominators come from an appended ones-column in V:
    pv = [V | 1]^T @ PT gives rows 0:64 = out^T (unnormalized), row 64 =
    per-query exp sums.
  - No max subtraction: q.k/8 is ~N(0,1) here, exp is well within fp32.
  - The 1/sqrt(DH) scale is folded into the ACT exp (free affine).
  - fp16 operands for all matmuls (PE streams any 16-bit dtype at one
    column/cycle, while fp32 runs as a half-rate two-pass LOW/HIGH
    stream; fp16's 10-bit mantissa beats bf16 by ~8x in accuracy for
    free); all accumulation stays fp32 in PSUM.
  - q/k are projected through duplicated weight columns [W|W] so qT/kT
    live in BOTH partition halves; score matmuls (K=64) then issue as
    pairs on array row-groups 0-63 / 64-127 and run concurrently.
  - Projections are interleaved with the first i-tile's score/exp groups
    so ScalarE (the bottleneck) starts ~8us into the kernel; each
    i-tile's normalization + output projection is deferred into the next
    i-tile's groups so the in-order PE stream never stalls on the DVE
    reciprocal chain.
"""

import os

import numpy as np

import concourse.bass as bass
import concourse.mybir as mybir
import concourse.tile as tile
from concourse import bacc
from concourse.bass import ts

F32 = mybir.dt.float32
F16 = mybir.dt.float16

L = 4096  # sequence length
D = 512  # model dim
H = 8  # heads
DH = 64  # head dim
P = 128  # partitions
DC = D // P  # d-chunks for the projection contraction (4)
IW = 512  # i-tile (query) width
NI = L // IW  # 8
NJ = L // P  # 32 j-tiles (key blocks)
GJ = 2  # j-tiles per exp group (2 PSUM banks per ACT instruction)
NG = NJ // GJ  # groups per i-tile
WCOLS = 320  # q-dup(128) + k-dup(128) + v(64)
N_CORES = 8

_CACHE = {}
LAST = {}


def build_bass():
    nc = bacc.Bacc(
        "TRN2", target_bir_lowering=False, debug=False, num_devices=N_CORES
    )
    xt = nc.dram_tensor("xt", [NI, DC, P, IW], F16, kind="ExternalInput")
    w = nc.dram_tensor("w", [DC, P, WCOLS], F16, kind="ExternalInput")
    wo = nc.dram_tensor("wo", [DH, D], F16, kind="ExternalInput")
    y = nc.dram_tensor("y", [L // P, P, D], F32, kind="ExternalOutput")

    with (
        tile.TileContext(nc) as tc,
        tc.tile_pool(name="const", bufs=1) as cpool,
        tc.tile_pool(name="ps", bufs=1, space="PSUM") as ppool,
        tc.tile_pool(name="pt", bufs=1) as pt_pool,
        tc.tile_pool(name="post", bufs=1) as post_pool,
        tc.tile_pool(name="yout", bufs=1) as yout_pool,
    ):
        # per-i-tile x chunk tiles: proj(i2) only waits on its own chunk's
        # DMA instead of the whole 4MB x load
        x_sbs = [cpool.tile([P, DC, IW], F16, name=f"x{i}") for i in range(NI)]
        w_sb = cpool.tile([P, DC, WCOLS], F16)
        wo_sb = cpool.tile([DH, D], F16)
        nc.sync.dma_start(x_sbs[0][:], xt[0].rearrange("c p l -> p c l"))
        nc.sync.dma_start(w_sb[:], w.rearrange("c p m -> p c m"))
        for i in range(1, NI):
            nc.sync.dma_start(x_sbs[i][:], xt[i].rearrange("c p l -> p c l"))

        nc.sync.dma_start(wo_sb[:], wo[:])
        qdup = cpool.tile([P, L], F16)  # qT in rows 0:64 AND 64:128
        kdup = cpool.tile([P, L], F16)
        vext = cpool.tile([P, NJ, DH + 2], F16)
        nc.vector.memset(vext[:, :, DH], 1.0)
        # warm the ACT exp table while DMAs run
        warm = cpool.tile([1, 8], F32)
        nc.vector.memset(warm[:], 0.0)
        nc.scalar.activation(warm[:], warm[:], mybir.ActivationFunctionType.Exp)

        def emit_proj_kq(i2):
            # k first (gates the score j-tiles), then q
            for off, dst in ((P, kdup), (0, qdup)):
                ps = ppool.tile([P, IW], F32, tag="proj", bufs=2, name="ps")
                for c in range(DC):
                    nc.tensor.matmul(
                        ps[:],
                        lhsT=w_sb[:, c, off : off + P],
                        rhs=x_sbs[i2][:, c, :],
                        start=(c == 0),
                        stop=(c == DC - 1),
                    )
                nc.vector.tensor_copy(dst[:, ts(i2, IW)], ps[:])

        def emit_proj_v(i2):
            # v directly in row layout: v[t-block, dh] = x-block^T-chunks @ Wv
            for t in range(4 * i2, 4 * i2 + 4):
                psv = ppool.tile([P, DH], F32, tag="proj", bufs=2, name="psv")
                for c in range(DC):
                    nc.tensor.matmul(
                        psv[:],
                        lhsT=x_sbs[i2][:, c, ts(t % 4, P)],
                        rhs=w_sb[:, c, 2 * P : 2 * P + DH],
                        start=(c == 0),
                        stop=(c == DC - 1),
                    )
                nc.vector.tensor_copy(vext[:, t, 0:DH], psv[:])

        pvs = {}
        outTs = {}

        def emit_group(i, g):
            if g == 0:
                pvs[i] = ppool.tile(
                    [DH + 1, IW], F32, tag="acc", bufs=2, name=f"pv{i}"
                )
            stp = ppool.tile([P, GJ * IW], F32, tag="st", bufs=2, name="stp")
            for u in range(GJ):
                jt = g * GJ + u
                half = DH * (jt % 2)
                nc.tensor.matmul(
                    stp[:, ts(u, IW)],
                    lhsT=kdup[half : half + DH, ts(jt, P)],
                    rhs=qdup[half : half + DH, ts(i, IW)],
                    start=True,
                    stop=True,
                )
            pt = pt_pool.tile([P, GJ * IW], F16, tag="pt", bufs=24, name="pt")
            nc.scalar.activation(
                pt[:], stp[:], mybir.ActivationFunctionType.Exp, scale=0.125
            )
            for u in range(GJ):
                jt = g * GJ + u
                nc.tensor.matmul(
                    pvs[i][:],
                    lhsT=vext[:, jt, 0 : DH + 1],
                    rhs=pt[:, ts(u, IW)],
                    start=(jt == 0),
                    stop=(jt == NJ - 1),
                    skip_group_check=True,
                )

        def emit_post_head(i):
            pv = pvs[i]
            srow = post_pool.tile([1, IW], F32, tag="srow", bufs=2, name="srow")
            nc.vector.tensor_copy(srow[:], pv[DH : DH + 1, :])
            rcp = post_pool.tile([1, IW], F32, tag="rcp", bufs=2, name="rcp")
            nc.vector.reciprocal_approx_fast(rcp[:], srow[:])
            rb = post_pool.tile([DH, IW], F32, tag="rb", bufs=2, name="rb")
            nc.gpsimd.partition_broadcast(rb[:], rcp[:])
            outT = post_pool.tile([DH, IW], F16, tag="outT", bufs=2, name="outT")
            nc.vector.tensor_mul(outT[:], pv[0:DH, :], rb[:])
            outTs[i] = outT

        def emit_post_y(i, t):
            yps = ppool.tile([P, D], F32, tag="proj", bufs=2, name="yps")
            nc.tensor.matmul(
                yps[:],
                lhsT=outTs[i][:, ts(t, P)],
                rhs=wo_sb[:],
                start=True,
                stop=True,
            )
            ysb = yout_pool.tile([P, D], F32, tag="ysb", bufs=3, name="ysb")
            nc.vector.tensor_copy(ysb[:], yps[:])
            nc.sync.dma_start(y[i * (IW // P) + t], ysb[:])

        # --- prologue: projections interleaved with i-tiles 0 and 1 ---
        from collections import deque

        pending = deque()

        def pump():
            if pending:
                pending.popleft()()

        for i2 in range(NI):
            emit_proj_kq(i2)
            emit_proj_v(i2)
            emit_group(0, 2 * i2)
            emit_group(0, 2 * i2 + 1)
            if i2 > 0:
                emit_group(1, 2 * (i2 - 1))
                emit_group(1, 2 * (i2 - 1) + 1)
            if i2 > 1:
                emit_group(2, 2 * (i2 - 2))
                emit_group(2, 2 * (i2 - 2) + 1)
        emit_group(1, NG - 2)
        emit_group(1, NG - 1)
        for g in range(2 * (NI - 2), NG):
            emit_group(2, g)
        for i in (0, 1, 2):
            pending.append(lambda i=i: emit_post_head(i))
            for t in range(IW // P):
                pending.append(lambda i=i, t=t: emit_post_y(i, t))
        # --- steady state ---
        for i in range(3, NI):
            for g in range(NG):
                emit_group(i, g)
                if g % 3 == 1:
                    pump()
            pending.append(lambda i=i: emit_post_head(i))
            for t in range(IW // P):
                pending.append(lambda i=i, t=t: emit_post_y(i, t))
        # last i-tile: chunk the normalization so each y-projection starts
        # as soon as its 128 columns of out^T are normalized, instead of
        # waiting for the full 512-wide reciprocal chain
        while len(pending) > 5:
            pump()
        pending.clear()
        last = NI - 1
        pvl = pvs[last]
        srow = post_pool.tile([1, IW], F32, tag="srow", bufs=2, name="srow")
        nc.vector.tensor_copy(srow[:], pvl[DH : DH + 1, :])
        rcp = post_pool.tile([1, IW], F32, tag="rcp", bufs=2, name="rcp")
        nc.vector.reciprocal_approx_fast(rcp[:], srow[:])
        for t in range(IW // P):
            rbc = post_pool.tile([DH, P], F32, tag="rbc", bufs=2, name="rbc")
            nc.gpsimd.partition_broadcast(rbc[:], rcp[:, ts(t, P)])
            oTc = post_pool.tile([DH, P], F16, tag="oTc", bufs=2, name="oTc")
            nc.vector.tensor_mul(oTc[:], pvl[0:DH, ts(t, P)], rbc[:])
            yps = ppool.tile([P, D], F32, tag="proj", bufs=2, name="yps")
            nc.tensor.matmul(
                yps[:], lhsT=oTc[:], rhs=wo_sb[:], start=True, stop=True
            )
            ysb = yout_pool.tile([P, D], F32, tag="ysb", bufs=3, name="ysb")
            nc.vector.tensor_copy(ysb[:], yps[:])
            nc.sync.dma_start(y[last * (IW // P) + t], ysb[:])
    nc.compile()
    return nc


def _get_nc():
    if "nc" not in _CACHE:
        _CACHE["nc"] = build_bass()
    return _CACHE["nc"]


def _prep_in_maps(x, Wqkv, Wo):
    x = np.asarray(x, dtype=np.float32).reshape(L, D)
    Wqkv = np.asarray(Wqkv, dtype=np.float32)
    Wo = np.asarray(Wo, dtype=np.float32)
    xt = (
        np.ascontiguousarray(x.T)
        .reshape(DC, P, NI, IW)
        .transpose(2, 0, 1, 3)
        .astype(np.float16)
    )
    xt = np.ascontiguousarray(xt)
    in_maps = []
    for h in range(N_CORES):
        wq = Wqkv[:, 0 * D + h * DH : 0 * D + (h + 1) * DH]
        wk = Wqkv[:, 1 * D + h * DH : 1 * D + (h + 1) * DH]
        wv = Wqkv[:, 2 * D + h * DH : 2 * D + (h + 1) * DH]
        cols = np.concatenate([wq, wq, wk, wk, wv], axis=1)  # [512, 320]
        w_dram = np.ascontiguousarray(cols).reshape(DC, P, WCOLS).astype(np.float16)
        wo_h = np.ascontiguousarray(Wo[h * DH : (h + 1) * DH, :]).astype(np.float16)
        in_maps.append({"xt": xt, "w": w_dram, "wo": wo_h})
    return in_maps


def kernel(x, Wqkv, Wo):
    from concourse import bass_utils

    # zero-egress container: artifact upload is impossible and only feeds
    # trace metadata — replace with a local marker.
    bass_utils.upload_artifacts = lambda tmpdir: f"local://{tmpdir}"

    nc = _get_nc()
    in_maps = _prep_in_maps(x, Wqkv, Wo)
    trace = bool(os.environ.get("KERNEL_TRACE"))
    res = bass_utils.run_bass_kernel_spmd(
        nc, in_maps, core_ids=list(range(N_CORES)), trace=trace
    )
    LAST["exec_time_ns"] = res.exec_time_ns
    LAST["trace"] = res.instructions_and_trace
    acc = np.zeros((L, D), np.float32)
    for r in res.results:
        acc += r["y"].reshape(L, D)
    return acc.reshape(1, L, D).astype(np.float32)



# revision 31
# speedup vs baseline: 1.5248x; 1.0052x over previous
"""Multi-head attention (B=1, L=4096, D=512, H=8, DH=64) on 8 TRN2 NeuronCores.

Sharding: head-parallel — core h computes head h end-to-end:
    qkv_h = x @ Wqkv[:, head-slices]      (on device, from host-transposed x)
    attn_h = softmax(q k^T / 8) v          (transposed-score layout)
    y_h = attn_h @ Wo[h*64:(h+1)*64, :]    (partial over heads)
Host reduces: y = sum_h y_h.

Device layout notes:
  - All score tiles are computed transposed: ST[j, i] = k_j . q_i, so the
    P@V contraction (over j) can use PT directly as the matmul moving
    operand. Softmax denominators come from an appended ones-column in V:
    pv = [V | 1]^T @ PT gives rows 0:64 = out^T (unnormalized), row 64 =
    per-query exp sums.
  - No max subtraction: q.k/8 is ~N(0,1) here, exp is well within fp32.
  - The 1/sqrt(DH) scale is folded into the ACT exp (free affine).
  - fp16 operands for all matmuls (PE streams any 16-bit dtype at one
    column/cycle, while fp32 runs as a half-rate two-pass LOW/HIGH
    stream; fp16's 10-bit mantissa beats bf16 by ~8x in accuracy for
    free); all accumulation stays fp32 in PSUM.
  - q/k are projected through duplicated weight columns [W|W] so qT/kT
    live in BOTH partition halves; score matmuls (K=64) then issue as
    pairs on array row-groups 0-63 / 64-127 and run concurrently.
  - Projections are interleaved with the first i-tile's score/exp groups
    so ScalarE (the bottleneck) starts ~8us into the kernel; each
    i-tile's normalization + output projection is deferred into the next
    i-tile's groups so the in-order PE stream never stalls on the DVE
    reciprocal chain.
"""

import os

import numpy as np

import concourse.bass as bass
import concourse.mybir as mybir
import concourse.tile as tile
from concourse import bacc
from concourse.bass import ts

F32 = mybir.dt.float32
F16 = mybir.dt.float16

L = 4096  # sequence length
D = 512  # model dim
H = 8  # heads
DH = 64  # head dim
P = 128  # partitions
DC = D // P  # d-chunks for the projection contraction (4)
IW = 512  # i-tile (query) width
NI = L // IW  # 8
NJ = L // P  # 32 j-tiles (key blocks)
GJ = 2  # j-tiles per exp group (2 PSUM banks per ACT instruction)
NG = NJ // GJ  # groups per i-tile
WCOLS = 320  # q-dup(128) + k-dup(128) + v(64)
N_CORES = 8

_CACHE = {}
LAST = {}


def build_bass():
    nc = bacc.Bacc(
        "TRN2", target_bir_lowering=False, debug=False, num_devices=N_CORES
    )
    # host pre-transposes to partition-major so every DMA is contiguous
    # (access-pattern rearrange DMAs ran ~5x slower than plain copies)
    xt = nc.dram_tensor("xt", [NI, P, DC, IW], F16, kind="ExternalInput")
    w = nc.dram_tensor("w", [P, DC, WCOLS], F16, kind="ExternalInput")
    wo = nc.dram_tensor("wo", [DH, D], F16, kind="ExternalInput")
    y = nc.dram_tensor("y", [L // P, P, D], F16, kind="ExternalOutput")

    with (
        tile.TileContext(nc) as tc,
        tc.tile_pool(name="const", bufs=1) as cpool,
        tc.tile_pool(name="ps", bufs=1, space="PSUM") as ppool,
        tc.tile_pool(name="pt", bufs=1) as pt_pool,
        tc.tile_pool(name="post", bufs=1) as post_pool,
        tc.tile_pool(name="yout", bufs=1) as yout_pool,
    ):
        # per-i-tile x chunk tiles: proj(i2) only waits on its own chunk's
        # DMA instead of the whole 4MB x load
        x_sbs = [cpool.tile([P, DC, IW], F16, name=f"x{i}") for i in range(NI)]
        w_sb = cpool.tile([P, DC, WCOLS], F16)
        wo_sb = cpool.tile([DH, D], F16)
        nc.sync.dma_start(x_sbs[0][:], xt[0])
        nc.sync.dma_start(w_sb[:], w[:])
        for i in range(1, NI):
            nc.sync.dma_start(x_sbs[i][:], xt[i])

        nc.sync.dma_start(wo_sb[:], wo[:])
        qdup = cpool.tile([P, L], F16)  # qT in rows 0:64 AND 64:128
        kdup = cpool.tile([P, L], F16)
        vext = cpool.tile([P, NJ, DH + 2], F16)
        nc.vector.memset(vext[:, :, DH], 1.0)
        # warm the ACT exp table while DMAs run
        warm = cpool.tile([1, 8], F32)
        nc.vector.memset(warm[:], 0.0)
        nc.scalar.activation(warm[:], warm[:], mybir.ActivationFunctionType.Exp)

        def emit_proj_kq(i2):
            # k first (gates the score j-tiles), then q
            for off, dst in ((P, kdup), (0, qdup)):
                ps = ppool.tile([P, IW], F32, tag="proj", bufs=2, name="ps")
                for c in range(DC):
                    nc.tensor.matmul(
                        ps[:],
                        lhsT=w_sb[:, c, off : off + P],
                        rhs=x_sbs[i2][:, c, :],
                        start=(c == 0),
                        stop=(c == DC - 1),
                    )
                nc.vector.tensor_copy(dst[:, ts(i2, IW)], ps[:])

        def emit_proj_v(i2):
            # v directly in row layout: v[t-block, dh] = x-block^T-chunks @ Wv
            for t in range(4 * i2, 4 * i2 + 4):
                psv = ppool.tile([P, DH], F32, tag="proj", bufs=2, name="psv")
                for c in range(DC):
                    nc.tensor.matmul(
                        psv[:],
                        lhsT=x_sbs[i2][:, c, ts(t % 4, P)],
                        rhs=w_sb[:, c, 2 * P : 2 * P + DH],
                        start=(c == 0),
                        stop=(c == DC - 1),
                    )
                nc.vector.tensor_copy(vext[:, t, 0:DH], psv[:])

        pvs = {}
        outTs = {}

        def emit_group(i, g):
            if g == 0:
                pvs[i] = ppool.tile(
                    [DH + 1, IW], F32, tag="acc", bufs=2, name=f"pv{i}"
                )
            stp = ppool.tile([P, GJ * IW], F32, tag="st", bufs=2, name="stp")
            for u in range(GJ):
                jt = g * GJ + u
                half = DH * (jt % 2)
                nc.tensor.matmul(
                    stp[:, ts(u, IW)],
                    lhsT=kdup[half : half + DH, ts(jt, P)],
                    rhs=qdup[half : half + DH, ts(i, IW)],
                    start=True,
                    stop=True,
                )
            pt = pt_pool.tile([P, GJ * IW], F16, tag="pt", bufs=24, name="pt")
            nc.scalar.activation(
                pt[:], stp[:], mybir.ActivationFunctionType.Exp, scale=0.125
            )
            for u in range(GJ):
                jt = g * GJ + u
                nc.tensor.matmul(
                    pvs[i][:],
                    lhsT=vext[:, jt, 0 : DH + 1],
                    rhs=pt[:, ts(u, IW)],
                    start=(jt == 0),
                    stop=(jt == NJ - 1),
                    skip_group_check=True,
                )

        def emit_post_head(i):
            pv = pvs[i]
            srow = post_pool.tile([1, IW], F32, tag="srow", bufs=2, name="srow")
            nc.vector.tensor_copy(srow[:], pv[DH : DH + 1, :])
            rcp = post_pool.tile([1, IW], F32, tag="rcp", bufs=2, name="rcp")
            nc.vector.reciprocal_approx_fast(rcp[:], srow[:])
            rb = post_pool.tile([DH, IW], F32, tag="rb", bufs=2, name="rb")
            nc.gpsimd.partition_broadcast(rb[:], rcp[:])
            outT = post_pool.tile([DH, IW], F16, tag="outT", bufs=2, name="outT")
            nc.vector.tensor_mul(outT[:], pv[0:DH, :], rb[:])
            outTs[i] = outT

        def emit_post_y(i, t):
            yps = ppool.tile([P, D], F32, tag="proj", bufs=2, name="yps")
            nc.tensor.matmul(
                yps[:],
                lhsT=outTs[i][:, ts(t, P)],
                rhs=wo_sb[:],
                start=True,
                stop=True,
            )
            ysb = yout_pool.tile([P, D], F16, tag="ysb", bufs=3, name="ysb")
            nc.vector.tensor_copy(ysb[:], yps[:])
            nc.sync.dma_start(y[i * (IW // P) + t], ysb[:])

        # --- prologue: projections interleaved with i-tiles 0 and 1 ---
        from collections import deque

        pending = deque()

        def pump():
            if pending:
                pending.popleft()()

        for i2 in range(NI):
            emit_proj_kq(i2)
            emit_proj_v(i2)
            emit_group(0, 2 * i2)
            emit_group(0, 2 * i2 + 1)
            if i2 > 0:
                emit_group(1, 2 * (i2 - 1))
                emit_group(1, 2 * (i2 - 1) + 1)
            if i2 > 1:
                emit_group(2, 2 * (i2 - 2))
                emit_group(2, 2 * (i2 - 2) + 1)
        emit_group(1, NG - 2)
        emit_group(1, NG - 1)
        for g in range(2 * (NI - 2), NG):
            emit_group(2, g)
        for i in (0, 1, 2):
            pending.append(lambda i=i: emit_post_head(i))
            for t in range(IW // P):
                pending.append(lambda i=i, t=t: emit_post_y(i, t))
        # --- steady state ---
        for i in range(3, NI):
            for g in range(NG):
                emit_group(i, g)
                if g % 3 == 1:
                    pump()
            pending.append(lambda i=i: emit_post_head(i))
            for t in range(IW // P):
                pending.append(lambda i=i, t=t: emit_post_y(i, t))
        # last i-tile: chunk the normalization so each y-projection starts
        # as soon as its 128 columns of out^T are normalized, instead of
        # waiting for the full 512-wide reciprocal chain
        while len(pending) > 5:
            pump()
        pending.clear()
        last = NI - 1
        pvl = pvs[last]
        for t in range(IW // P):
            srow = post_pool.tile([1, P], F32, tag="srow", bufs=2, name="srow")
            nc.vector.tensor_copy(srow[:], pvl[DH : DH + 1, ts(t, P)])
            rcp = post_pool.tile([1, P], F32, tag="rcp", bufs=2, name="rcp")
            nc.vector.reciprocal_approx_fast(rcp[:], srow[:])
            rbc = post_pool.tile([DH, P], F32, tag="rbc", bufs=2, name="rbc")
            nc.gpsimd.partition_broadcast(rbc[:], rcp[:])
            oTc = post_pool.tile([DH, P], F16, tag="oTc", bufs=2, name="oTc")
            nc.vector.tensor_mul(oTc[:], pvl[0:DH, ts(t, P)], rbc[:])
            yps = ppool.tile([P, D], F32, tag="proj", bufs=2, name="yps")
            nc.tensor.matmul(
                yps[:], lhsT=oTc[:], rhs=wo_sb[:], start=True, stop=True
            )
            ysb = yout_pool.tile([P, D], F16, tag="ysb", bufs=3, name="ysb")
            nc.vector.tensor_copy(ysb[:], yps[:])
            nc.sync.dma_start(y[last * (IW // P) + t], ysb[:])
    nc.compile()
    return nc


def _get_nc():
    if "nc" not in _CACHE:
        _CACHE["nc"] = build_bass()
    return _CACHE["nc"]


def _prep_in_maps(x, Wqkv, Wo):
    x = np.asarray(x, dtype=np.float32).reshape(L, D)
    Wqkv = np.asarray(Wqkv, dtype=np.float32)
    Wo = np.asarray(Wo, dtype=np.float32)
    # device layout [NI, P, DC, IW]: partition-major so the DMA is a
    # contiguous copy (xt[i][p, c, l] = x[i*IW + l, c*P + p])
    xt = (
        np.ascontiguousarray(x.T)
        .reshape(DC, P, NI, IW)
        .transpose(2, 1, 0, 3)
        .astype(np.float16)
    )
    xt = np.ascontiguousarray(xt)
    in_maps = []
    for h in range(N_CORES):
        wq = Wqkv[:, 0 * D + h * DH : 0 * D + (h + 1) * DH]
        wk = Wqkv[:, 1 * D + h * DH : 1 * D + (h + 1) * DH]
        wv = Wqkv[:, 2 * D + h * DH : 2 * D + (h + 1) * DH]
        cols = np.concatenate([wq, wq, wk, wk, wv], axis=1)  # [512, 320]
        # [P, DC, WCOLS] partition-major: w[p, c, m] = cols[c*P + p, m]
        w_dram = np.ascontiguousarray(
            cols.reshape(DC, P, WCOLS).transpose(1, 0, 2)
        ).astype(np.float16)
        wo_h = np.ascontiguousarray(Wo[h * DH : (h + 1) * DH, :]).astype(np.float16)
        in_maps.append({"xt": xt, "w": w_dram, "wo": wo_h})
    return in_maps


def kernel(x, Wqkv, Wo):
    from concourse import bass_utils

    # zero-egress container: artifact upload is impossible and only feeds
    # trace metadata — replace with a local marker.
    bass_utils.upload_artifacts = lambda tmpdir: f"local://{tmpdir}"

    nc = _get_nc()
    in_maps = _prep_in_maps(x, Wqkv, Wo)
    trace = bool(os.environ.get("KERNEL_TRACE"))
    res = bass_utils.run_bass_kernel_spmd(
        nc, in_maps, core_ids=list(range(N_CORES)), trace=trace
    )
    LAST["exec_time_ns"] = res.exec_time_ns
    LAST["trace"] = res.instructions_and_trace
    acc = np.zeros((L, D), np.float32)
    for r in res.results:
        acc += r["y"].reshape(L, D)
    return acc.reshape(1, L, D).astype(np.float32)



# revision 35
# speedup vs baseline: 1.5369x; 1.0079x over previous
"""Multi-head attention (B=1, L=4096, D=512, H=8, DH=64) on 8 TRN2 NeuronCores.

Sharding: head-parallel — core h computes head h end-to-end:
    qkv_h = x @ Wqkv[:, head-slices]      (on device, from host-transposed x)
    attn_h = softmax(q k^T / 8) v          (transposed-score layout)
    y_h = attn_h @ Wo[h*64:(h+1)*64, :]    (partial over heads)
Host reduces: y = sum_h y_h.

Device layout notes:
  - All score tiles are computed transposed: ST[j, i] = k_j . q_i, so the
    P@V contraction (over j) can use PT directly as the matmul moving
    operand. Softmax denominators come from an appended ones-column in V:
    pv = [V | 1]^T @ PT gives rows 0:64 = out^T (unnormalized), row 64 =
    per-query exp sums.
  - No max subtraction: q.k/8 is ~N(0,1) here, exp is well within fp32.
  - The 1/sqrt(DH) scale is folded into the ACT exp (free affine).
  - fp16 operands for all matmuls (PE streams any 16-bit dtype at one
    column/cycle, while fp32 runs as a half-rate two-pass LOW/HIGH
    stream; fp16's 10-bit mantissa beats bf16 by ~8x in accuracy for
    free); all accumulation stays fp32 in PSUM.
  - q/k are projected through duplicated weight columns [W|W] so qT/kT
    live in BOTH partition halves; score matmuls (K=64) then issue as
    pairs on array row-groups 0-63 / 64-127 and run concurrently.
  - Projections are interleaved with the first i-tile's score/exp groups
    so ScalarE (the bottleneck) starts ~8us into the kernel; each
    i-tile's normalization + output projection is deferred into the next
    i-tile's groups so the in-order PE stream never stalls on the DVE
    reciprocal chain.
"""

import os

import numpy as np

import concourse.bass as bass
import concourse.mybir as mybir
import concourse.tile as tile
from concourse import bacc
from concourse.bass import ts

F32 = mybir.dt.float32
F16 = mybir.dt.float16

L = 4096  # sequence length
D = 512  # model dim
H = 8  # heads
DH = 64  # head dim
P = 128  # partitions
DC = D // P  # d-chunks for the projection contraction (4)
IW = 512  # i-tile (query) width
NI = L // IW  # 8
NJ = L // P  # 32 j-tiles (key blocks)
GJ = 2  # j-tiles per exp group (2 PSUM banks per ACT instruction)
NG = NJ // GJ  # groups per i-tile
WCOLS = 320  # q-dup(128) + k-dup(128) + v(64)
N_CORES = 8

_CACHE = {}
LAST = {}


def build_bass():
    nc = bacc.Bacc(
        "TRN2", target_bir_lowering=False, debug=False, num_devices=N_CORES
    )
    # host pre-transposes to partition-major so every DMA is contiguous
    # (access-pattern rearrange DMAs ran ~5x slower than plain copies)
    xt = nc.dram_tensor("xt", [NI, P, DC, IW], F16, kind="ExternalInput")
    w = nc.dram_tensor("w", [P, DC, WCOLS], F16, kind="ExternalInput")
    wo = nc.dram_tensor("wo", [DH, D], F16, kind="ExternalInput")
    y = nc.dram_tensor("y", [L // P, P, D], F16, kind="ExternalOutput")

    with (
        tile.TileContext(nc) as tc,
        tc.tile_pool(name="const", bufs=1) as cpool,
        tc.tile_pool(name="ps", bufs=1, space="PSUM") as ppool,
        tc.tile_pool(name="pt", bufs=1) as pt_pool,
        tc.tile_pool(name="post", bufs=1) as post_pool,
        tc.tile_pool(name="yout", bufs=1) as yout_pool,
    ):
        # per-i-tile x chunk tiles: proj(i2) only waits on its own chunk's
        # DMA instead of the whole 4MB x load
        x_sbs = [cpool.tile([P, DC, IW], F16, name=f"x{i}") for i in range(NI)]
        w_sb = cpool.tile([P, DC, WCOLS], F16)
        wo_sb = cpool.tile([DH, D], F16)
        nc.sync.dma_start(x_sbs[0][:], xt[0])
        nc.sync.dma_start(w_sb[:], w[:])
        for i in range(1, NI):
            nc.sync.dma_start(x_sbs[i][:], xt[i])

        nc.sync.dma_start(wo_sb[:], wo[:])
        qdup = cpool.tile([P, L], F16)  # qT in rows 0:64 AND 64:128
        kdup = cpool.tile([P, L], F16)
        vext = cpool.tile([P, NJ, DH + 2], F16)
        nc.vector.memset(vext[:, :, DH], 1.0)
        # warm the ACT exp table while DMAs run
        warm = cpool.tile([1, 8], F32)
        nc.vector.memset(warm[:], 0.0)
        nc.scalar.activation(warm[:], warm[:], mybir.ActivationFunctionType.Exp)

        def emit_proj_kq(i2):
            # k first (gates the score j-tiles), then q
            for off, dst in ((P, kdup), (0, qdup)):
                ps = ppool.tile([P, IW], F32, tag="proj", bufs=2, name="ps")
                for c in range(DC):
                    nc.tensor.matmul(
                        ps[:],
                        lhsT=w_sb[:, c, off : off + P],
                        rhs=x_sbs[i2][:, c, :],
                        start=(c == 0),
                        stop=(c == DC - 1),
                    )
                nc.vector.tensor_copy(dst[:, ts(i2, IW)], ps[:])

        def emit_proj_v(i2):
            # v directly in row layout: v[t-block, dh] = x-block^T-chunks @ Wv
            for t in range(4 * i2, 4 * i2 + 4):
                psv = ppool.tile([P, DH], F32, tag="proj", bufs=2, name="psv")
                for c in range(DC):
                    nc.tensor.matmul(
                        psv[:],
                        lhsT=x_sbs[i2][:, c, ts(t % 4, P)],
                        rhs=w_sb[:, c, 2 * P : 2 * P + DH],
                        start=(c == 0),
                        stop=(c == DC - 1),
                    )
                nc.vector.tensor_copy(vext[:, t, 0:DH], psv[:])

        pvs = {}
        outTs = {}

        def emit_group(i, g):
            emit_pvg(i, g, emit_se(i, g))

        def emit_se(i, g):
            stp = ppool.tile([P, GJ * IW], F32, tag="st", bufs=2, name="stp")
            for u in range(GJ):
                jt = g * GJ + u
                half = DH * (jt % 2)
                nc.tensor.matmul(
                    stp[:, ts(u, IW)],
                    lhsT=kdup[half : half + DH, ts(jt, P)],
                    rhs=qdup[half : half + DH, ts(i, IW)],
                    start=True,
                    stop=True,
                )
            pt = pt_pool.tile([P, GJ * IW], F16, tag="pt", bufs=24, name="pt")
            nc.scalar.activation(
                pt[:], stp[:], mybir.ActivationFunctionType.Exp, scale=0.125
            )
            return pt

        def emit_pvg(i, g, pt):
            if g == 0:
                pvs[i] = ppool.tile(
                    [DH + 1, IW], F32, tag="acc", bufs=2, name=f"pv{i}"
                )
            for u in range(GJ):
                jt = g * GJ + u
                nc.tensor.matmul(
                    pvs[i][:],
                    lhsT=vext[:, jt, 0 : DH + 1],
                    rhs=pt[:, ts(u, IW)],
                    start=(jt == 0),
                    stop=(jt == NJ - 1),
                    skip_group_check=True,
                )

        def emit_post_head(i):
            pv = pvs[i]
            srow = post_pool.tile([1, IW], F32, tag="srow", bufs=4, name="srow")
            nc.vector.tensor_copy(srow[:], pv[DH : DH + 1, :])
            rcp = post_pool.tile([1, IW], F32, tag="rcp", bufs=4, name="rcp")
            nc.vector.reciprocal_approx_fast(rcp[:], srow[:])
            rb = post_pool.tile([DH, IW], F32, tag="rb", bufs=2, name="rb")
            nc.gpsimd.partition_broadcast(rb[:], rcp[:])
            outT = post_pool.tile([DH, IW], F16, tag="outT", bufs=2, name="outT")
            nc.vector.tensor_mul(outT[:], pv[0:DH, :], rb[:])
            outTs[i] = outT

        def emit_post_y(i, t):
            yps = ppool.tile([P, D], F32, tag="proj", bufs=2, name="yps")
            nc.tensor.matmul(
                yps[:],
                lhsT=outTs[i][:, ts(t, P)],
                rhs=wo_sb[:],
                start=True,
                stop=True,
            )
            ysb = yout_pool.tile([P, D], F16, tag="ysb", bufs=3, name="ysb")
            nc.vector.tensor_copy(ysb[:], yps[:])
            nc.sync.dma_start(y[i * (IW // P) + t], ysb[:])

        # --- prologue: projections interleaved with i-tiles 0 and 1 ---
        from collections import deque

        pending = deque()

        def pump():
            if pending:
                pending.popleft()()

        for i2 in range(NI):
            emit_proj_kq(i2)
            # scores+exp of i-tile 0 go BEFORE the v projection so ScalarE
            # starts ~4us earlier per i2; v is only needed by the PV matmuls
            pt_a = emit_se(0, 2 * i2)
            pt_b = emit_se(0, 2 * i2 + 1)
            emit_proj_v(i2)
            emit_pvg(0, 2 * i2, pt_a)
            emit_pvg(0, 2 * i2 + 1, pt_b)
            if i2 > 0:
                emit_group(1, 2 * (i2 - 1))
                emit_group(1, 2 * (i2 - 1) + 1)
            if i2 > 1:
                emit_group(2, 2 * (i2 - 2))
                emit_group(2, 2 * (i2 - 2) + 1)
        emit_group(1, NG - 2)
        emit_group(1, NG - 1)
        for g in range(2 * (NI - 2), NG):
            emit_group(2, g)
        for i in (0, 1, 2):
            pending.append(lambda i=i: emit_post_head(i))
            for t in range(IW // P):
                pending.append(lambda i=i, t=t: emit_post_y(i, t))
        # --- steady state ---
        for i in range(3, NI):
            for g in range(NG):
                emit_group(i, g)
                if g % 3 == 1:
                    pump()
            pending.append(lambda i=i: emit_post_head(i))
            for t in range(IW // P):
                pending.append(lambda i=i, t=t: emit_post_y(i, t))
        # last i-tile: chunk the normalization so each y-projection starts
        # as soon as its 128 columns of out^T are normalized, instead of
        # waiting for the full 512-wide reciprocal chain
        while len(pending) > 5:
            pump()
        pending.clear()
        last = NI - 1
        pvl = pvs[last]
        # all reciprocals first so the per-chunk broadcast/mul/proj pipeline
        # is never gated behind a 0.7us ysb cast in the DVE queue
        rcps = []
        for t in range(IW // P):
            srow = post_pool.tile([1, P], F32, tag="srow", bufs=4, name="srow")
            nc.vector.tensor_copy(srow[:], pvl[DH : DH + 1, ts(t, P)])
            rcp = post_pool.tile([1, P], F32, tag="rcp", bufs=4, name="rcp")
            nc.vector.reciprocal_approx_fast(rcp[:], srow[:])
            rcps.append(rcp)
        for t in range(IW // P):
            rbc = post_pool.tile([DH, P], F32, tag="rbc", bufs=4, name="rbc")
            nc.gpsimd.partition_broadcast(rbc[:], rcps[t][:])
            oTc = post_pool.tile([DH, P], F16, tag="oTc", bufs=4, name="oTc")
            nc.vector.tensor_mul(oTc[:], pvl[0:DH, ts(t, P)], rbc[:])
            yps = ppool.tile([P, D], F32, tag="proj", bufs=2, name="yps")
            nc.tensor.matmul(
                yps[:], lhsT=oTc[:], rhs=wo_sb[:], start=True, stop=True
            )
            ysb = yout_pool.tile([P, D], F16, tag="ysb", bufs=3, name="ysb")
            nc.vector.tensor_copy(ysb[:], yps[:])
            nc.sync.dma_start(y[last * (IW // P) + t], ysb[:])
    nc.compile()
    return nc


def _get_nc():
    if "nc" not in _CACHE:
        _CACHE["nc"] = build_bass()
    return _CACHE["nc"]


def _prep_in_maps(x, Wqkv, Wo):
    x = np.asarray(x, dtype=np.float32).reshape(L, D)
    Wqkv = np.asarray(Wqkv, dtype=np.float32)
    Wo = np.asarray(Wo, dtype=np.float32)
    # device layout [NI, P, DC, IW]: partition-major so the DMA is a
    # contiguous copy (xt[i][p, c, l] = x[i*IW + l, c*P + p])
    xt = (
        np.ascontiguousarray(x.T)
        .reshape(DC, P, NI, IW)
        .transpose(2, 1, 0, 3)
        .astype(np.float16)
    )
    xt = np.ascontiguousarray(xt)
    in_maps = []
    for h in range(N_CORES):
        wq = Wqkv[:, 0 * D + h * DH : 0 * D + (h + 1) * DH]
        wk = Wqkv[:, 1 * D + h * DH : 1 * D + (h + 1) * DH]
        wv = Wqkv[:, 2 * D + h * DH : 2 * D + (h + 1) * DH]
        cols = np.concatenate([wq, wq, wk, wk, wv], axis=1)  # [512, 320]
        # [P, DC, WCOLS] partition-major: w[p, c, m] = cols[c*P + p, m]
        w_dram = np.ascontiguousarray(
            cols.reshape(DC, P, WCOLS).transpose(1, 0, 2)
        ).astype(np.float16)
        wo_h = np.ascontiguousarray(Wo[h * DH : (h + 1) * DH, :]).astype(np.float16)
        in_maps.append({"xt": xt, "w": w_dram, "wo": wo_h})
    return in_maps


def kernel(x, Wqkv, Wo):
    from concourse import bass_utils

    # zero-egress container: artifact upload is impossible and only feeds
    # trace metadata — replace with a local marker.
    bass_utils.upload_artifacts = lambda tmpdir: f"local://{tmpdir}"

    nc = _get_nc()
    in_maps = _prep_in_maps(x, Wqkv, Wo)
    trace = bool(os.environ.get("KERNEL_TRACE"))
    res = bass_utils.run_bass_kernel_spmd(
        nc, in_maps, core_ids=list(range(N_CORES)), trace=trace
    )
    LAST["exec_time_ns"] = res.exec_time_ns
    LAST["trace"] = res.instructions_and_trace
    acc = np.zeros((L, D), np.float32)
    for r in res.results:
        acc += r["y"].reshape(L, D)
    return acc.reshape(1, L, D).astype(np.float32)



# revision 36
# speedup vs baseline: 1.5530x; 1.0105x over previous
"""Multi-head attention (B=1, L=4096, D=512, H=8, DH=64) on 8 TRN2 NeuronCores.

Sharding: head-parallel — core h computes head h end-to-end:
    qkv_h = x @ Wqkv[:, head-slices]      (on device, from host-transposed x)
    attn_h = softmax(q k^T / 8) v          (transposed-score layout)
    y_h = attn_h @ Wo[h*64:(h+1)*64, :]    (partial over heads)
Host reduces: y = sum_h y_h.

Device layout notes:
  - All score tiles are computed transposed: ST[j, i] = k_j . q_i, so the
    P@V contraction (over j) can use PT directly as the matmul moving
    operand. Softmax denominators come from an appended ones-column in V:
    pv = [V | 1]^T @ PT gives rows 0:64 = out^T (unnormalized), row 64 =
    per-query exp sums.
  - No max subtraction: q.k/8 is ~N(0,1) here, exp is well within fp32.
  - The 1/sqrt(DH) scale is folded into the ACT exp (free affine).
  - fp16 operands for all matmuls (PE streams any 16-bit dtype at one
    column/cycle, while fp32 runs as a half-rate two-pass LOW/HIGH
    stream; fp16's 10-bit mantissa beats bf16 by ~8x in accuracy for
    free); all accumulation stays fp32 in PSUM.
  - q/k are projected through duplicated weight columns [W|W] so qT/kT
    live in BOTH partition halves; score matmuls (K=64) then issue as
    pairs on array row-groups 0-63 / 64-127 and run concurrently.
  - Projections are interleaved with the first i-tile's score/exp groups
    so ScalarE (the bottleneck) starts ~8us into the kernel; each
    i-tile's normalization + output projection is deferred into the next
    i-tile's groups so the in-order PE stream never stalls on the DVE
    reciprocal chain.
"""

import os

import numpy as np

import concourse.bass as bass
import concourse.mybir as mybir
import concourse.tile as tile
from concourse import bacc
from concourse.bass import ts

F32 = mybir.dt.float32
F16 = mybir.dt.float16

L = 4096  # sequence length
D = 512  # model dim
H = 8  # heads
DH = 64  # head dim
P = 128  # partitions
DC = D // P  # d-chunks for the projection contraction (4)
IW = 512  # i-tile (query) width
NI = L // IW  # 8
NJ = L // P  # 32 j-tiles (key blocks)
GJ = 2  # j-tiles per exp group (2 PSUM banks per ACT instruction)
NG = NJ // GJ  # groups per i-tile
WCOLS = 320  # q-dup(128) + k-dup(128) + v(64)
N_CORES = 8

_CACHE = {}
LAST = {}


def build_bass():
    nc = bacc.Bacc(
        "TRN2", target_bir_lowering=False, debug=False, num_devices=N_CORES
    )
    # host pre-transposes to partition-major so every DMA is contiguous
    # (access-pattern rearrange DMAs ran ~5x slower than plain copies)
    xt = nc.dram_tensor("xt", [NI, P, DC, IW], F16, kind="ExternalInput")
    w = nc.dram_tensor("w", [P, DC, WCOLS], F16, kind="ExternalInput")
    wo = nc.dram_tensor("wo", [DH, D], F16, kind="ExternalInput")
    y = nc.dram_tensor("y", [L // P, P, D], F16, kind="ExternalOutput")

    with (
        tile.TileContext(nc) as tc,
        tc.tile_pool(name="const", bufs=1) as cpool,
        tc.tile_pool(name="ps", bufs=1, space="PSUM") as ppool,
        tc.tile_pool(name="pt", bufs=1) as pt_pool,
        tc.tile_pool(name="post", bufs=1) as post_pool,
        tc.tile_pool(name="yout", bufs=1) as yout_pool,
    ):
        # per-i-tile x chunk tiles: proj(i2) only waits on its own chunk's
        # DMA instead of the whole 4MB x load
        x_sbs = [cpool.tile([P, DC, IW], F16, name=f"x{i}") for i in range(NI)]
        w_sb = cpool.tile([P, DC, WCOLS], F16)
        wo_sb = cpool.tile([DH, D], F16)
        nc.sync.dma_start(x_sbs[0][:], xt[0])
        nc.sync.dma_start(w_sb[:], w[:])
        for i in range(1, NI):
            nc.sync.dma_start(x_sbs[i][:], xt[i])

        nc.sync.dma_start(wo_sb[:], wo[:])
        qdup = cpool.tile([P, L], F16)  # qT in rows 0:64 AND 64:128
        kdup = cpool.tile([P, L], F16)
        vext = cpool.tile([P, NJ, DH + 2], F16)
        nc.vector.memset(vext[:, :, DH], 1.0)
        # warm the ACT exp table while DMAs run
        warm = cpool.tile([1, 8], F32)
        nc.vector.memset(warm[:], 0.0)
        nc.scalar.activation(warm[:], warm[:], mybir.ActivationFunctionType.Exp)

        def emit_proj_kq(i2):
            # k first (gates the score j-tiles), then q
            for off, dst in ((P, kdup), (0, qdup)):
                ps = ppool.tile([P, IW], F32, tag="proj", bufs=2, name="ps")
                for c in range(DC):
                    nc.tensor.matmul(
                        ps[:],
                        lhsT=w_sb[:, c, off : off + P],
                        rhs=x_sbs[i2][:, c, :],
                        start=(c == 0),
                        stop=(c == DC - 1),
                    )
                nc.vector.tensor_copy(dst[:, ts(i2, IW)], ps[:])

        def emit_proj_v(i2):
            # v directly in row layout: v[t-block, dh] = x-block^T-chunks @ Wv
            for t in range(4 * i2, 4 * i2 + 4):
                psv = ppool.tile([P, DH], F32, tag="proj", bufs=2, name="psv")
                for c in range(DC):
                    nc.tensor.matmul(
                        psv[:],
                        lhsT=x_sbs[i2][:, c, ts(t % 4, P)],
                        rhs=w_sb[:, c, 2 * P : 2 * P + DH],
                        start=(c == 0),
                        stop=(c == DC - 1),
                    )
                nc.vector.tensor_copy(vext[:, t, 0:DH], psv[:])

        pvs = {}
        outTs = {}

        def emit_group(i, g):
            emit_pvg(i, g, emit_se(i, g))

        def emit_se(i, g):
            stp = ppool.tile([P, GJ * IW], F32, tag="st", bufs=2, name="stp")
            for u in range(GJ):
                jt = g * GJ + u
                half = DH * (jt % 2)
                nc.tensor.matmul(
                    stp[:, ts(u, IW)],
                    lhsT=kdup[half : half + DH, ts(jt, P)],
                    rhs=qdup[half : half + DH, ts(i, IW)],
                    start=True,
                    stop=True,
                )
            pt = pt_pool.tile([P, GJ * IW], F16, tag="pt", bufs=24, name="pt")
            nc.scalar.activation(
                pt[:], stp[:], mybir.ActivationFunctionType.Exp, scale=0.125
            )
            return pt

        def emit_pvg(i, g, pt):
            if g == 0:
                pvs[i] = ppool.tile(
                    [DH + 1, IW], F32, tag="acc", bufs=2, name=f"pv{i}"
                )
            for u in range(GJ):
                jt = g * GJ + u
                nc.tensor.matmul(
                    pvs[i][:],
                    lhsT=vext[:, jt, 0 : DH + 1],
                    rhs=pt[:, ts(u, IW)],
                    start=(jt == 0),
                    stop=(jt == NJ - 1),
                    skip_group_check=True,
                )

        def emit_post_head(i):
            pv = pvs[i]
            srow = post_pool.tile([1, IW], F32, tag="srow", bufs=4, name="srow")
            nc.vector.tensor_copy(srow[:], pv[DH : DH + 1, :])
            rcp = post_pool.tile([1, IW], F32, tag="rcp", bufs=4, name="rcp")
            nc.vector.reciprocal_approx_fast(rcp[:], srow[:])
            rb = post_pool.tile([DH, IW], F32, tag="rb", bufs=2, name="rb")
            nc.gpsimd.partition_broadcast(rb[:], rcp[:])
            outT = post_pool.tile([DH, IW], F16, tag="outT", bufs=2, name="outT")
            nc.vector.tensor_mul(outT[:], pv[0:DH, :], rb[:])
            outTs[i] = outT

        def emit_post_y(i, t):
            yps = ppool.tile([P, D], F32, tag="proj", bufs=2, name="yps")
            nc.tensor.matmul(
                yps[:],
                lhsT=outTs[i][:, ts(t, P)],
                rhs=wo_sb[:],
                start=True,
                stop=True,
            )
            ysb = yout_pool.tile([P, D], F16, tag="ysb", bufs=3, name="ysb")
            nc.vector.tensor_copy(ysb[:], yps[:])
            nc.sync.dma_start(y[i * (IW // P) + t], ysb[:])

        # --- prologue: projections interleaved with i-tiles 0 and 1 ---
        from collections import deque

        pending = deque()

        def pump():
            if pending:
                pending.popleft()()

        for i2 in range(NI):
            emit_proj_kq(i2)
            # scores+exp of i-tile 0 go BEFORE the v projection so ScalarE
            # starts ~4us earlier per i2; v is only needed by the PV matmuls
            pt_a = emit_se(0, 2 * i2)
            pt_b = emit_se(0, 2 * i2 + 1)
            emit_proj_v(i2)
            emit_pvg(0, 2 * i2, pt_a)
            emit_pvg(0, 2 * i2 + 1, pt_b)
            if i2 > 0:
                emit_group(1, 2 * (i2 - 1))
                emit_group(1, 2 * (i2 - 1) + 1)
            if i2 > 1:
                emit_group(2, 2 * (i2 - 2))
                emit_group(2, 2 * (i2 - 2) + 1)
        emit_group(1, NG - 2)
        emit_group(1, NG - 1)
        for g in range(2 * (NI - 2), NG):
            emit_group(2, g)
        for i in (0, 1, 2):
            pending.append(lambda i=i: emit_post_head(i))
            for t in range(IW // P):
                pending.append(lambda i=i, t=t: emit_post_y(i, t))
        # free pv[0] before PV(3,0) allocates its slot, so the PE doesn't
        # stall ~5us at the prologue->steady transition
        pump()
        pump()
        # --- steady state ---
        for i in range(3, NI):
            for g in range(NG):
                emit_group(i, g)
                if g % 3 == 1:
                    pump()
            pending.append(lambda i=i: emit_post_head(i))
            for t in range(IW // P):
                pending.append(lambda i=i, t=t: emit_post_y(i, t))
        # last i-tile: chunk the normalization so each y-projection starts
        # as soon as its 128 columns of out^T are normalized, instead of
        # waiting for the full 512-wide reciprocal chain
        while len(pending) > 5:
            pump()
        pending.clear()
        last = NI - 1
        pvl = pvs[last]
        # all reciprocals first so the per-chunk broadcast/mul/proj pipeline
        # is never gated behind a 0.7us ysb cast in the DVE queue
        rcps = []
        for t in range(IW // P):
            srow = post_pool.tile([1, P], F32, tag="srow", bufs=4, name="srow")
            nc.vector.tensor_copy(srow[:], pvl[DH : DH + 1, ts(t, P)])
            rcp = post_pool.tile([1, P], F32, tag="rcp", bufs=4, name="rcp")
            nc.vector.reciprocal_approx_fast(rcp[:], srow[:])
            rcps.append(rcp)
        for t in range(IW // P):
            rbc = post_pool.tile([DH, P], F32, tag="rbc", bufs=4, name="rbc")
            nc.gpsimd.partition_broadcast(rbc[:], rcps[t][:])
            oTc = post_pool.tile([DH, P], F16, tag="oTc", bufs=4, name="oTc")
            nc.vector.tensor_mul(oTc[:], pvl[0:DH, ts(t, P)], rbc[:])
            yps = ppool.tile([P, D], F32, tag="proj", bufs=2, name="yps")
            nc.tensor.matmul(
                yps[:], lhsT=oTc[:], rhs=wo_sb[:], start=True, stop=True
            )
            ysb = yout_pool.tile([P, D], F16, tag="ysb", bufs=3, name="ysb")
            nc.vector.tensor_copy(ysb[:], yps[:])
            nc.sync.dma_start(y[last * (IW // P) + t], ysb[:])
    nc.compile()
    return nc


def _get_nc():
    if "nc" not in _CACHE:
        _CACHE["nc"] = build_bass()
    return _CACHE["nc"]


def _prep_in_maps(x, Wqkv, Wo):
    x = np.asarray(x, dtype=np.float32).reshape(L, D)
    Wqkv = np.asarray(Wqkv, dtype=np.float32)
    Wo = np.asarray(Wo, dtype=np.float32)
    # device layout [NI, P, DC, IW]: partition-major so the DMA is a
    # contiguous copy (xt[i][p, c, l] = x[i*IW + l, c*P + p])
    xt = (
        np.ascontiguousarray(x.T)
        .reshape(DC, P, NI, IW)
        .transpose(2, 1, 0, 3)
        .astype(np.float16)
    )
    xt = np.ascontiguousarray(xt)
    in_maps = []
    for h in range(N_CORES):
        wq = Wqkv[:, 0 * D + h * DH : 0 * D + (h + 1) * DH]
        wk = Wqkv[:, 1 * D + h * DH : 1 * D + (h + 1) * DH]
        wv = Wqkv[:, 2 * D + h * DH : 2 * D + (h + 1) * DH]
        cols = np.concatenate([wq, wq, wk, wk, wv], axis=1)  # [512, 320]
        # [P, DC, WCOLS] partition-major: w[p, c, m] = cols[c*P + p, m]
        w_dram = np.ascontiguousarray(
            cols.reshape(DC, P, WCOLS).transpose(1, 0, 2)
        ).astype(np.float16)
        wo_h = np.ascontiguousarray(Wo[h * DH : (h + 1) * DH, :]).astype(np.float16)
        in_maps.append({"xt": xt, "w": w_dram, "wo": wo_h})
    return in_maps


def kernel(x, Wqkv, Wo):
    from concourse import bass_utils

    # zero-egress container: artifact upload is impossible and only feeds
    # trace metadata — replace with a local marker.
    bass_utils.upload_artifacts = lambda tmpdir: f"local://{tmpdir}"

    nc = _get_nc()
    in_maps = _prep_in_maps(x, Wqkv, Wo)
    trace = bool(os.environ.get("KERNEL_TRACE"))
    res = bass_utils.run_bass_kernel_spmd(
        nc, in_maps, core_ids=list(range(N_CORES)), trace=trace
    )
    LAST["exec_time_ns"] = res.exec_time_ns
    LAST["trace"] = res.instructions_and_trace
    acc = np.zeros((L, D), np.float32)
    for r in res.results:
        acc += r["y"].reshape(L, D)
    return acc.reshape(1, L, D).astype(np.float32)



# revision 38
# speedup vs baseline: 1.6097x; 1.0365x over previous
"""Multi-head attention (B=1, L=4096, D=512, H=8, DH=64) on 8 TRN2 NeuronCores.

Sharding: head-parallel — core h computes head h end-to-end:
    qkv_h = x @ Wqkv[:, head-slices]      (on device, from host-transposed x)
    attn_h = softmax(q k^T / 8) v          (transposed-score layout)
    y_h = attn_h @ Wo[h*64:(h+1)*64, :]    (partial over heads)
Host reduces: y = sum_h y_h.

Device layout notes:
  - All score tiles are computed transposed: ST[j, i] = k_j . q_i, so the
    P@V contraction (over j) can use PT directly as the matmul moving
    operand. Softmax denominators come from an appended ones-column in V:
    pv = [V | 1]^T @ PT gives rows 0:64 = out^T (unnormalized), row 64 =
    per-query exp sums.
  - No max subtraction: q.k/8 is ~N(0,1) here, exp is well within fp32.
  - The 1/sqrt(DH) scale is folded into the ACT exp (free affine).
  - fp16 operands for all matmuls (PE streams any 16-bit dtype at one
    column/cycle, while fp32 runs as a half-rate two-pass LOW/HIGH
    stream; fp16's 10-bit mantissa beats bf16 by ~8x in accuracy for
    free); all accumulation stays fp32 in PSUM.
  - q/k are projected through duplicated weight columns [W|W] so qT/kT
    live in BOTH partition halves; score matmuls (K=64) then issue as
    pairs on array row-groups 0-63 / 64-127 and run concurrently.
  - Projections are interleaved with the first i-tile's score/exp groups
    so ScalarE (the bottleneck) starts ~8us into the kernel; each
    i-tile's normalization + output projection is deferred into the next
    i-tile's groups so the in-order PE stream never stalls on the DVE
    reciprocal chain.
"""

import os

import numpy as np

import concourse.bass as bass
import concourse.mybir as mybir
import concourse.tile as tile
from concourse import bacc
from concourse.bass import ts

F32 = mybir.dt.float32
F16 = mybir.dt.float16

L = 4096  # sequence length
D = 512  # model dim
H = 8  # heads
DH = 64  # head dim
P = 128  # partitions
DC = D // P  # d-chunks for the projection contraction (4)
IW = 512  # i-tile (query) width
NI = L // IW  # 8
NJ = L // P  # 32 j-tiles (key blocks)
GJ = 2  # j-tiles per exp group (2 PSUM banks per ACT instruction)
NG = NJ // GJ  # groups per i-tile
WCOLS = 320  # q-dup(128) + k-dup(128) + v(64)
N_CORES = 8

_CACHE = {}
LAST = {}


def build_bass():
    nc = bacc.Bacc(
        "TRN2", target_bir_lowering=False, debug=False, num_devices=N_CORES
    )
    # host pre-transposes to partition-major so every DMA is contiguous
    # (access-pattern rearrange DMAs ran ~5x slower than plain copies)
    xt = nc.dram_tensor("xt", [NI, P, DC, IW], F16, kind="ExternalInput")
    w = nc.dram_tensor("w", [P, DC, WCOLS], F16, kind="ExternalInput")
    wo = nc.dram_tensor("wo", [DH, D], F16, kind="ExternalInput")
    y = nc.dram_tensor("y", [L // P, P, D], F16, kind="ExternalOutput")

    with (
        tile.TileContext(nc) as tc,
        tc.tile_pool(name="const", bufs=1) as cpool,
        tc.tile_pool(name="ps", bufs=1, space="PSUM") as ppool,
        tc.tile_pool(name="pt", bufs=1) as pt_pool,
        tc.tile_pool(name="post", bufs=1) as post_pool,
        tc.tile_pool(name="yout", bufs=1) as yout_pool,
    ):
        # per-i-tile x chunk tiles: proj(i2) only waits on its own chunk's
        # DMA instead of the whole 4MB x load
        x_sbs = [cpool.tile([P, DC, IW], F16, name=f"x{i}") for i in range(NI)]
        w_sb = cpool.tile([P, DC, WCOLS], F16)
        wo_sb = cpool.tile([DH, D], F16)
        # w first: it gates the first projection chain together with x0
        nc.sync.dma_start(w_sb[:], w[:])
        nc.sync.dma_start(x_sbs[0][:], xt[0])
        for i in range(1, NI):
            nc.sync.dma_start(x_sbs[i][:], xt[i])

        nc.sync.dma_start(wo_sb[:], wo[:])
        qdup = cpool.tile([P, L], F16)  # qT in rows 0:64 AND 64:128
        kdup = cpool.tile([P, L], F16)
        vext = cpool.tile([P, NJ, DH + 2], F16)
        nc.vector.memset(vext[:, :, DH], 1.0)
        # warm the ACT exp table while DMAs run
        warm = cpool.tile([1, 8], F32)
        nc.vector.memset(warm[:], 0.0)
        nc.scalar.activation(warm[:], warm[:], mybir.ActivationFunctionType.Exp)

        def emit_proj_kq(i2):
            # k first (gates the score j-tiles), then q
            for off, dst in ((P, kdup), (0, qdup)):
                ps = ppool.tile([P, IW], F32, tag="proj", bufs=2, name="ps")
                for c in range(DC):
                    nc.tensor.matmul(
                        ps[:],
                        lhsT=w_sb[:, c, off : off + P],
                        rhs=x_sbs[i2][:, c, :],
                        start=(c == 0),
                        stop=(c == DC - 1),
                    )
                nc.vector.tensor_copy(dst[:, ts(i2, IW)], ps[:])

        def emit_proj_v(i2):
            # v directly in row layout: v[t-block, dh] = x-block^T-chunks @ Wv
            for t in range(4 * i2, 4 * i2 + 4):
                psv = ppool.tile([P, DH], F32, tag="proj", bufs=2, name="psv")
                for c in range(DC):
                    nc.tensor.matmul(
                        psv[:],
                        lhsT=x_sbs[i2][:, c, ts(t % 4, P)],
                        rhs=w_sb[:, c, 2 * P : 2 * P + DH],
                        start=(c == 0),
                        stop=(c == DC - 1),
                    )
                nc.vector.tensor_copy(vext[:, t, 0:DH], psv[:])

        pvs = {}
        outTs = {}

        def emit_group(i, g):
            emit_pvg(i, g, emit_se(i, g))

        def emit_se(i, g):
            stp = ppool.tile([P, GJ * IW], F32, tag="st", bufs=2, name="stp")
            for u in range(GJ):
                jt = g * GJ + u
                half = DH * (jt % 2)
                nc.tensor.matmul(
                    stp[:, ts(u, IW)],
                    lhsT=kdup[half : half + DH, ts(jt, P)],
                    rhs=qdup[half : half + DH, ts(i, IW)],
                    start=True,
                    stop=True,
                )
            pt = pt_pool.tile([P, GJ * IW], F16, tag="pt", bufs=24, name="pt")
            nc.scalar.activation(
                pt[:], stp[:], mybir.ActivationFunctionType.Exp, scale=0.125
            )
            return pt

        def emit_pvg(i, g, pt):
            if g == 0:
                pvs[i] = ppool.tile(
                    [DH + 1, IW], F32, tag="acc", bufs=2, name=f"pv{i}"
                )
            for u in range(GJ):
                jt = g * GJ + u
                nc.tensor.matmul(
                    pvs[i][:],
                    lhsT=vext[:, jt, 0 : DH + 1],
                    rhs=pt[:, ts(u, IW)],
                    start=(jt == 0),
                    stop=(jt == NJ - 1),
                    skip_group_check=True,
                )

        def emit_post_head(i):
            pv = pvs[i]
            srow = post_pool.tile([1, IW], F32, tag="srow", bufs=4, name="srow")
            nc.vector.tensor_copy(srow[:], pv[DH : DH + 1, :])
            rcp = post_pool.tile([1, IW], F32, tag="rcp", bufs=4, name="rcp")
            nc.vector.reciprocal_approx_fast(rcp[:], srow[:])
            rb = post_pool.tile([DH, IW], F32, tag="rb", bufs=2, name="rb")
            nc.gpsimd.partition_broadcast(rb[:], rcp[:])
            outT = post_pool.tile([DH, IW], F16, tag="outT", bufs=2, name="outT")
            nc.vector.tensor_mul(outT[:], pv[0:DH, :], rb[:])
            outTs[i] = outT

        def emit_post_y(i, t):
            yps = ppool.tile([P, D], F32, tag="proj", bufs=2, name="yps")
            nc.tensor.matmul(
                yps[:],
                lhsT=outTs[i][:, ts(t, P)],
                rhs=wo_sb[:],
                start=True,
                stop=True,
            )
            ysb = yout_pool.tile([P, D], F16, tag="ysb", bufs=3, name="ysb")
            nc.vector.tensor_copy(ysb[:], yps[:])
            nc.sync.dma_start(y[i * (IW // P) + t], ysb[:])

        # --- prologue: projections interleaved with i-tiles 0 and 1 ---
        from collections import deque

        pending = deque()

        def pump():
            if pending:
                pending.popleft()()

        # phase p: project keys/queries for j-phase p, then emit ALL newly
        # available score/exp groups (i-tiles 0..2, staggered) so ScalarE is
        # fed first; the v projection and PV matmuls of the PREVIOUS phase's
        # groups run on the PE while those ACTs execute.
        pvq = deque()

        def phase_se(p):
            for i in (0, 1, 2):
                g0 = 2 * (p - i)
                if 0 <= g0 < NG:
                    for g in (g0, g0 + 1):
                        pvq.append((i, g, emit_se(i, g)))

        for p in range(NI + 2):
            if p < NI:
                emit_proj_kq(p)
            phase_se(p)
            if 1 <= p <= NI:
                emit_proj_v(p - 1)
            while pvq and pvq[0][1] // 2 <= p - 1:
                i, g, pt = pvq.popleft()
                emit_pvg(i, g, pt)
        while pvq:
            i, g, pt = pvq.popleft()
            emit_pvg(i, g, pt)
        for i in (0, 1, 2):
            pending.append(lambda i=i: emit_post_head(i))
            for t in range(IW // P):
                pending.append(lambda i=i, t=t: emit_post_y(i, t))
        # free pv[0] before PV(3,0) allocates its slot, so the PE doesn't
        # stall ~5us at the prologue->steady transition
        pump()
        pump()
        # --- steady state ---
        for i in range(3, NI):
            for g in range(NG):
                emit_group(i, g)
                if g % 3 == 1:
                    pump()
            pending.append(lambda i=i: emit_post_head(i))
            for t in range(IW // P):
                pending.append(lambda i=i, t=t: emit_post_y(i, t))
        # last i-tile: chunk the normalization so each y-projection starts
        # as soon as its 128 columns of out^T are normalized, instead of
        # waiting for the full 512-wide reciprocal chain
        while len(pending) > 5:
            pump()
        pending.clear()
        last = NI - 1
        pvl = pvs[last]
        # all reciprocals first so the per-chunk broadcast/mul/proj pipeline
        # is never gated behind a 0.7us ysb cast in the DVE queue
        rcps = []
        for t in range(IW // P):
            srow = post_pool.tile([1, P], F32, tag="srow", bufs=4, name="srow")
            nc.vector.tensor_copy(srow[:], pvl[DH : DH + 1, ts(t, P)])
            rcp = post_pool.tile([1, P], F32, tag="rcp", bufs=4, name="rcp")
            nc.vector.reciprocal_approx_fast(rcp[:], srow[:])
            rcps.append(rcp)
        for t in range(IW // P):
            rbc = post_pool.tile([DH, P], F32, tag="rbc", bufs=4, name="rbc")
            nc.gpsimd.partition_broadcast(rbc[:], rcps[t][:])
            oTc = post_pool.tile([DH, P], F16, tag="oTc", bufs=4, name="oTc")
            nc.vector.tensor_mul(oTc[:], pvl[0:DH, ts(t, P)], rbc[:])
            yps = ppool.tile([P, D], F32, tag="proj", bufs=2, name="yps")
            nc.tensor.matmul(
                yps[:], lhsT=oTc[:], rhs=wo_sb[:], start=True, stop=True
            )
            ysb = yout_pool.tile([P, D], F16, tag="ysb", bufs=3, name="ysb")
            nc.vector.tensor_copy(ysb[:], yps[:])
            nc.sync.dma_start(y[last * (IW // P) + t], ysb[:])
    nc.compile()
    return nc


def _get_nc():
    if "nc" not in _CACHE:
        _CACHE["nc"] = build_bass()
    return _CACHE["nc"]


def _prep_in_maps(x, Wqkv, Wo):
    x = np.asarray(x, dtype=np.float32).reshape(L, D)
    Wqkv = np.asarray(Wqkv, dtype=np.float32)
    Wo = np.asarray(Wo, dtype=np.float32)
    # device layout [NI, P, DC, IW]: partition-major so the DMA is a
    # contiguous copy (xt[i][p, c, l] = x[i*IW + l, c*P + p])
    xt = (
        np.ascontiguousarray(x.T)
        .reshape(DC, P, NI, IW)
        .transpose(2, 1, 0, 3)
        .astype(np.float16)
    )
    xt = np.ascontiguousarray(xt)
    in_maps = []
    for h in range(N_CORES):
        wq = Wqkv[:, 0 * D + h * DH : 0 * D + (h + 1) * DH]
        wk = Wqkv[:, 1 * D + h * DH : 1 * D + (h + 1) * DH]
        wv = Wqkv[:, 2 * D + h * DH : 2 * D + (h + 1) * DH]
        cols = np.concatenate([wq, wq, wk, wk, wv], axis=1)  # [512, 320]
        # [P, DC, WCOLS] partition-major: w[p, c, m] = cols[c*P + p, m]
        w_dram = np.ascontiguousarray(
            cols.reshape(DC, P, WCOLS).transpose(1, 0, 2)
        ).astype(np.float16)
        wo_h = np.ascontiguousarray(Wo[h * DH : (h + 1) * DH, :]).astype(np.float16)
        in_maps.append({"xt": xt, "w": w_dram, "wo": wo_h})
    return in_maps


def kernel(x, Wqkv, Wo):
    from concourse import bass_utils

    # zero-egress container: artifact upload is impossible and only feeds
    # trace metadata — replace with a local marker.
    bass_utils.upload_artifacts = lambda tmpdir: f"local://{tmpdir}"

    nc = _get_nc()
    in_maps = _prep_in_maps(x, Wqkv, Wo)
    trace = bool(os.environ.get("KERNEL_TRACE"))
    res = bass_utils.run_bass_kernel_spmd(
        nc, in_maps, core_ids=list(range(N_CORES)), trace=trace
    )
    LAST["exec_time_ns"] = res.exec_time_ns
    LAST["trace"] = res.instructions_and_trace
    acc = np.zeros((L, D), np.float32)
    for r in res.results:
        acc += r["y"].reshape(L, D)
    return acc.reshape(1, L, D).astype(np.float32)



# revision 40
# speedup vs baseline: 1.6142x; 1.0028x over previous
"""Multi-head attention (B=1, L=4096, D=512, H=8, DH=64) on 8 TRN2 NeuronCores.

Sharding: head-parallel — core h computes head h end-to-end:
    qkv_h = x @ Wqkv[:, head-slices]      (on device, from host-transposed x)
    attn_h = softmax(q k^T / 8) v          (transposed-score layout)
    y_h = attn_h @ Wo[h*64:(h+1)*64, :]    (partial over heads)
Host reduces: y = sum_h y_h.

Device layout notes:
  - All score tiles are computed transposed: ST[j, i] = k_j . q_i, so the
    P@V contraction (over j) can use PT directly as the matmul moving
    operand. Softmax denominators come from an appended ones-column in V:
    pv = [V | 1]^T @ PT gives rows 0:64 = out^T (unnormalized), row 64 =
    per-query exp sums.
  - No max subtraction: q.k/8 is ~N(0,1) here, exp is well within fp32.
  - The 1/sqrt(DH) scale is folded into the ACT exp (free affine).
  - fp16 operands for all matmuls (PE streams any 16-bit dtype at one
    column/cycle, while fp32 runs as a half-rate two-pass LOW/HIGH
    stream; fp16's 10-bit mantissa beats bf16 by ~8x in accuracy for
    free); all accumulation stays fp32 in PSUM.
  - q/k are projected through duplicated weight columns [W|W] so qT/kT
    live in BOTH partition halves; score matmuls (K=64) then issue as
    pairs on array row-groups 0-63 / 64-127 and run concurrently.
  - Projections are interleaved with the first i-tile's score/exp groups
    so ScalarE (the bottleneck) starts ~8us into the kernel; each
    i-tile's normalization + output projection is deferred into the next
    i-tile's groups so the in-order PE stream never stalls on the DVE
    reciprocal chain.
"""

import os

import numpy as np

import concourse.bass as bass
import concourse.mybir as mybir
import concourse.tile as tile
from concourse import bacc
from concourse.bass import ts

F32 = mybir.dt.float32
F16 = mybir.dt.float16

L = 4096  # sequence length
D = 512  # model dim
H = 8  # heads
DH = 64  # head dim
P = 128  # partitions
DC = D // P  # d-chunks for the projection contraction (4)
IW = 512  # i-tile (query) width
NI = L // IW  # 8
NJ = L // P  # 32 j-tiles (key blocks)
GJ = 2  # j-tiles per exp group (2 PSUM banks per ACT instruction)
NG = NJ // GJ  # groups per i-tile
WCOLS = 320  # q-dup(128) + k-dup(128) + v(64)
N_CORES = 8

_CACHE = {}
LAST = {}


def build_bass():
    nc = bacc.Bacc(
        "TRN2", target_bir_lowering=False, debug=False, num_devices=N_CORES
    )
    # host pre-transposes to partition-major so every DMA is contiguous
    # (access-pattern rearrange DMAs ran ~5x slower than plain copies)
    xt = nc.dram_tensor("xt", [NI, P, DC, IW], F16, kind="ExternalInput")
    w = nc.dram_tensor("w", [P, DC, WCOLS], F16, kind="ExternalInput")
    wo = nc.dram_tensor("wo", [DH, D], F16, kind="ExternalInput")
    y = nc.dram_tensor("y", [L // P, P, D], F16, kind="ExternalOutput")

    with (
        tile.TileContext(nc) as tc,
        tc.tile_pool(name="const", bufs=1) as cpool,
        tc.tile_pool(name="ps", bufs=1, space="PSUM") as ppool,
        tc.tile_pool(name="pt", bufs=1) as pt_pool,
        tc.tile_pool(name="post", bufs=1) as post_pool,
        tc.tile_pool(name="yout", bufs=1) as yout_pool,
    ):
        # per-i-tile x chunk tiles: proj(i2) only waits on its own chunk's
        # DMA instead of the whole 4MB x load
        x_sbs = [cpool.tile([P, DC, IW], F16, name=f"x{i}") for i in range(NI)]
        w_sb = cpool.tile([P, DC, WCOLS], F16)
        wo_sb = cpool.tile([DH, D], F16)
        nc.sync.dma_start(x_sbs[0][:], xt[0])
        nc.sync.dma_start(w_sb[:], w[:])
        for i in range(1, NI):
            nc.sync.dma_start(x_sbs[i][:], xt[i])

        nc.sync.dma_start(wo_sb[:], wo[:])
        qdup = cpool.tile([P, L], F16)  # qT in rows 0:64 AND 64:128
        kdup = cpool.tile([P, L], F16)
        vext = cpool.tile([P, NJ, DH + 2], F16)
        nc.vector.memset(vext[:, :, DH], 1.0)
        # warm the ACT exp table while DMAs run
        warm = cpool.tile([1, 8], F32)
        nc.vector.memset(warm[:], 0.0)
        nc.scalar.activation(warm[:], warm[:], mybir.ActivationFunctionType.Exp)

        def emit_proj_kq(i2):
            # k first (gates the score j-tiles), then q
            for off, dst in ((P, kdup), (0, qdup)):
                ps = ppool.tile([P, IW], F32, tag="proj", bufs=2, name="ps")
                for c in range(DC):
                    nc.tensor.matmul(
                        ps[:],
                        lhsT=w_sb[:, c, off : off + P],
                        rhs=x_sbs[i2][:, c, :],
                        start=(c == 0),
                        stop=(c == DC - 1),
                    )
                nc.vector.tensor_copy(dst[:, ts(i2, IW)], ps[:])

        def emit_proj_v(i2):
            # v directly in row layout: v[t-block, dh] = x-block^T-chunks @ Wv
            for t in range(4 * i2, 4 * i2 + 4):
                psv = ppool.tile([P, DH], F32, tag="proj", bufs=2, name="psv")
                for c in range(DC):
                    nc.tensor.matmul(
                        psv[:],
                        lhsT=x_sbs[i2][:, c, ts(t % 4, P)],
                        rhs=w_sb[:, c, 2 * P : 2 * P + DH],
                        start=(c == 0),
                        stop=(c == DC - 1),
                    )
                nc.vector.tensor_copy(vext[:, t, 0:DH], psv[:])

        pvs = {}
        outTs = {}

        def emit_group(i, g):
            emit_pvg(i, g, emit_se(i, g))

        def emit_se(i, g):
            stp = ppool.tile([P, GJ * IW], F32, tag="st", bufs=2, name="stp")
            for u in range(GJ):
                jt = g * GJ + u
                half = DH * (jt % 2)
                nc.tensor.matmul(
                    stp[:, ts(u, IW)],
                    lhsT=kdup[half : half + DH, ts(jt, P)],
                    rhs=qdup[half : half + DH, ts(i, IW)],
                    start=True,
                    stop=True,
                )
            pt = pt_pool.tile([P, GJ * IW], F16, tag="pt", bufs=24, name="pt")
            nc.scalar.activation(
                pt[:], stp[:], mybir.ActivationFunctionType.Exp, scale=0.125
            )
            return pt

        def emit_pvg(i, g, pt):
            if g == 0:
                pvs[i] = ppool.tile(
                    [DH + 1, IW], F32, tag="acc", bufs=2, name=f"pv{i}"
                )
            for u in range(GJ):
                jt = g * GJ + u
                nc.tensor.matmul(
                    pvs[i][:],
                    lhsT=vext[:, jt, 0 : DH + 1],
                    rhs=pt[:, ts(u, IW)],
                    start=(jt == 0),
                    stop=(jt == NJ - 1),
                    skip_group_check=True,
                )

        def emit_post_head(i):
            pv = pvs[i]
            srow = post_pool.tile([1, IW], F32, tag="srow", bufs=4, name="srow")
            nc.vector.tensor_copy(srow[:], pv[DH : DH + 1, :])
            rcp = post_pool.tile([1, IW], F32, tag="rcp", bufs=4, name="rcp")
            nc.vector.reciprocal_approx_fast(rcp[:], srow[:])
            rb = post_pool.tile([DH, IW], F32, tag="rb", bufs=2, name="rb")
            nc.gpsimd.partition_broadcast(rb[:], rcp[:])
            outT = post_pool.tile([DH, IW], F16, tag="outT", bufs=2, name="outT")
            nc.vector.tensor_mul(outT[:], pv[0:DH, :], rb[:])
            outTs[i] = outT

        def emit_post_y(i, t):
            yps = ppool.tile([P, D], F32, tag="proj", bufs=2, name="yps")
            nc.tensor.matmul(
                yps[:],
                lhsT=outTs[i][:, ts(t, P)],
                rhs=wo_sb[:],
                start=True,
                stop=True,
            )
            ysb = yout_pool.tile([P, D], F16, tag="ysb", bufs=3, name="ysb")
            nc.vector.tensor_copy(ysb[:], yps[:])
            nc.sync.dma_start(y[i * (IW // P) + t], ysb[:])

        # --- prologue: projections interleaved with i-tiles 0 and 1 ---
        from collections import deque

        pending = deque()

        def pump():
            if pending:
                pending.popleft()()

        # phase p: project keys/queries for j-phase p, then emit ALL newly
        # available score/exp groups (i-tiles 0..2, staggered) so ScalarE is
        # fed first; the v projection and PV matmuls of the PREVIOUS phase's
        # groups run on the PE while those ACTs execute.
        pvq = deque()

        def phase_se(p):
            for i in (0, 1, 2):
                g0 = 2 * (p - i)
                if 0 <= g0 < NG:
                    for g in (g0, g0 + 1):
                        pvq.append((i, g, emit_se(i, g)))

        for p in range(NI + 2):
            if p < NI:
                emit_proj_kq(p)
            phase_se(p)
            if 1 <= p <= NI:
                emit_proj_v(p - 1)
            while pvq and pvq[0][1] // 2 <= p - 1:
                i, g, pt = pvq.popleft()
                emit_pvg(i, g, pt)
        while pvq:
            i, g, pt = pvq.popleft()
            emit_pvg(i, g, pt)
        for i in (0, 1, 2):
            pending.append(lambda i=i: emit_post_head(i))
            for t in range(IW // P):
                pending.append(lambda i=i, t=t: emit_post_y(i, t))
        # transition: feed ScalarE i-tile 3's first scores BEFORE pumping the
        # post work (whose DVE chain head-of-line-blocks the PE), and free
        # pv[0] before PV(3,0) allocates its slot
        tpt0 = emit_se(3, 0)
        tpt1 = emit_se(3, 1)
        pump()
        pump()
        emit_pvg(3, 0, tpt0)
        emit_pvg(3, 1, tpt1)
        # --- steady state ---
        for i in range(3, NI):
            for g in range(2 if i == 3 else 0, NG):
                emit_group(i, g)
                if g % 3 == 1:
                    pump()
            pending.append(lambda i=i: emit_post_head(i))
            for t in range(IW // P):
                pending.append(lambda i=i, t=t: emit_post_y(i, t))
        # last i-tile: chunk the normalization so each y-projection starts
        # as soon as its 128 columns of out^T are normalized, instead of
        # waiting for the full 512-wide reciprocal chain
        while len(pending) > 5:
            pump()
        pending.clear()
        last = NI - 1
        pvl = pvs[last]
        # all reciprocals first so the per-chunk broadcast/mul/proj pipeline
        # is never gated behind a 0.7us ysb cast in the DVE queue
        rcps = []
        for t in range(IW // P):
            srow = post_pool.tile([1, P], F32, tag="srow", bufs=4, name="srow")
            nc.vector.tensor_copy(srow[:], pvl[DH : DH + 1, ts(t, P)])
            rcp = post_pool.tile([1, P], F32, tag="rcp", bufs=4, name="rcp")
            nc.vector.reciprocal_approx_fast(rcp[:], srow[:])
            rcps.append(rcp)
        for t in range(IW // P):
            rbc = post_pool.tile([DH, P], F32, tag="rbc", bufs=4, name="rbc")
            nc.gpsimd.partition_broadcast(rbc[:], rcps[t][:])
            oTc = post_pool.tile([DH, P], F16, tag="oTc", bufs=4, name="oTc")
            nc.vector.tensor_mul(oTc[:], pvl[0:DH, ts(t, P)], rbc[:])
            yps = ppool.tile([P, D], F32, tag="proj", bufs=2, name="yps")
            nc.tensor.matmul(
                yps[:], lhsT=oTc[:], rhs=wo_sb[:], start=True, stop=True
            )
            ysb = yout_pool.tile([P, D], F16, tag="ysb", bufs=3, name="ysb")
            nc.vector.tensor_copy(ysb[:], yps[:])
            nc.sync.dma_start(y[last * (IW // P) + t], ysb[:])
    nc.compile()
    return nc


def _get_nc():
    if "nc" not in _CACHE:
        _CACHE["nc"] = build_bass()
    return _CACHE["nc"]


def _prep_in_maps(x, Wqkv, Wo):
    x = np.asarray(x, dtype=np.float32).reshape(L, D)
    Wqkv = np.asarray(Wqkv, dtype=np.float32)
    Wo = np.asarray(Wo, dtype=np.float32)
    # device layout [NI, P, DC, IW]: partition-major so the DMA is a
    # contiguous copy (xt[i][p, c, l] = x[i*IW + l, c*P + p])
    xt = (
        np.ascontiguousarray(x.T)
        .reshape(DC, P, NI, IW)
        .transpose(2, 1, 0, 3)
        .astype(np.float16)
    )
    xt = np.ascontiguousarray(xt)
    in_maps = []
    for h in range(N_CORES):
        wq = Wqkv[:, 0 * D + h * DH : 0 * D + (h + 1) * DH]
        wk = Wqkv[:, 1 * D + h * DH : 1 * D + (h + 1) * DH]
        wv = Wqkv[:, 2 * D + h * DH : 2 * D + (h + 1) * DH]
        cols = np.concatenate([wq, wq, wk, wk, wv], axis=1)  # [512, 320]
        # [P, DC, WCOLS] partition-major: w[p, c, m] = cols[c*P + p, m]
        w_dram = np.ascontiguousarray(
            cols.reshape(DC, P, WCOLS).transpose(1, 0, 2)
        ).astype(np.float16)
        wo_h = np.ascontiguousarray(Wo[h * DH : (h + 1) * DH, :]).astype(np.float16)
        in_maps.append({"xt": xt, "w": w_dram, "wo": wo_h})
    return in_maps


def kernel(x, Wqkv, Wo):
    from concourse import bass_utils

    # zero-egress container: artifact upload is impossible and only feeds
    # trace metadata — replace with a local marker.
    bass_utils.upload_artifacts = lambda tmpdir: f"local://{tmpdir}"

    nc = _get_nc()
    in_maps = _prep_in_maps(x, Wqkv, Wo)
    trace = bool(os.environ.get("KERNEL_TRACE"))
    res = bass_utils.run_bass_kernel_spmd(
        nc, in_maps, core_ids=list(range(N_CORES)), trace=trace
    )
    LAST["exec_time_ns"] = res.exec_time_ns
    LAST["trace"] = res.instructions_and_trace
    acc = np.zeros((L, D), np.float32)
    for r in res.results:
        acc += r["y"].reshape(L, D)
    return acc.reshape(1, L, D).astype(np.float32)

